# revision 29
# baseline (speedup 1.0000x reference)
"""Trainium2 Bass kernel for batched multi-head self-attention.

Reference computation (per batch element b):
    qkv = x @ w_qkv.T                  # [N, 3C]
    q, k, v = split/reshape to heads   # H=16 heads, d=64
    attn = softmax(q @ k.T / sqrt(d))
    out = (attn @ v) reshaped back     # [N, C]
    y = out @ w_proj.T + b_proj

Sharding: pure data-parallel over batch B=8 across the 8 NeuronCores
(one batch element per core, weights replicated, no collectives).

On-device layout (everything transposed so matmuls contract over the
partition axis with no input transposes):
  - xT      [C, N]   (host pre-transposed, bf16)
  - wqkT    [C, 2C]  (host pre-transposed q,k columns, bf16)
  - wprojT  [C, C]   (host pre-transposed, bf16)

Cost model: a PE matmul costs out_free_size * 0.4167ns regardless of
contraction size or output partitions.  The key restructure vs the
229us baseline is the attn@V matmul orientation:
  - scores S^T [m,n] tiles (K=64, head pairs row-packed): free=512.
  - AV is computed FLIPPED: out[n, d+1] = pt[m, n-tile].T @ [V | 1]
    with free=65 instead of the old [d+1, 512] free=512 layout.
    This halves AV PE time (54.6us -> 27.7us).  The ones column gives
    softmax row sums per output partition, so normalization is a
    per-partition tensor_scalar divide on DVE (no partition
    broadcasts).
  - The [n, c] result is transposed back to [c, n] for the output
    projection with PE transpose instructions (128x128 via identity,
    53ns each, 64 total = 3.4us).
  - exp() is done on [128, 1024] head-pair tiles (one ACT instruction
    per (n2, m)) to amortize the ~185ns ACT access-latency overhead;
    ACT total 133us stays under the PE's ~197us.
Emission interleaves each attention slot (pr, n2) with the previous
slot's AV/transpose work and the next pair's q/k projection chains so
the PE never waits on ACT; the output projection fills the tail.
PSUM budget (8 banks): st 2x2 + acc 1 + av 2 + tr 1.
"""

import os
import sys

for _p in ("/opt/trn_rl_repo", "/root/.axon_site/_ro/trn_rl_repo"):
    if os.path.isdir(_p) and _p not in sys.path:
        sys.path.insert(0, _p)
        break

import numpy as np
import ml_dtypes

import concourse.bass as bass
import concourse.bacc as bacc
import concourse.tile as tile
import concourse.mybir as mybir
from concourse import bass_utils

BF16 = mybir.dt.bfloat16
F32 = mybir.dt.float32
AF = mybir.ActivationFunctionType
ALU = mybir.AluOpType

B, N, C, H = 8, 1024, 1024, 16
D = C // H            # 64 head dim
P = 128               # partitions
CT = C // P           # 8 contraction tiles
NT2 = N // 512        # 2 n-tiles of 512
MT = N // P           # 8 m-tiles of 128
PAIRS = H // 2        # 8 head pairs
SCALE = float(D) ** -0.5
N_CORES = 8

_cache = {}


def _build():
    nc = bacc.Bacc("TRN2", target_bir_lowering=False, debug=False,
                   enable_asserts=False, num_devices=N_CORES)

    xw_d = nc.dram_tensor("xw", [C, 2 * N], BF16, kind="ExternalInput")
    wqkT_d = nc.dram_tensor("wqkT", [C, 2 * C], BF16, kind="ExternalInput")
    wprojT_d = nc.dram_tensor("wprojT", [C, C], BF16, kind="ExternalInput")
    bias_d = nc.dram_tensor("bias", [P, CT], F32, kind="ExternalInput")
    ident_d = nc.dram_tensor("ident", [P, P], BF16, kind="ExternalInput")
    outT_d = nc.dram_tensor("outT", [C, N], BF16, kind="ExternalOutput")

    with tile.TileContext(nc) as tc:
        with (
            tc.tile_pool(name="res", bufs=1) as rp,
            tc.tile_pool(name="work", bufs=2) as wp,
            tc.tile_pool(name="ps", bufs=1, space="PSUM") as pp,
        ):
            # ---------------- PE warm-up ----------------
            # Cover the initial input-DMA wait with dummy matmuls so the
            # p-state/HAM ramp completes before real work arrives.
            warm_a = wp.tile([P, 512], BF16, name="warm_a", tag="warm_a",
                             bufs=1)
            nc.vector.memset(warm_a[:], 0.25)
            warm_ps = pp.tile([P, 512], F32, name="warm_ps", tag="acc",
                              bufs=1)
            for _ in range(5):
                nc.tensor.matmul(warm_ps[:], warm_a[:, 0:P], warm_a[:],
                                 start=True, stop=True)

            # ---------------- resident inputs ----------------
            xT = []
            wqv = []
            for i in range(CT):
                t = rp.tile([P, 2 * N], BF16, name=f"xw{i}", tag=f"xw{i}")
                nc.sync.dma_start(t[:], xw_d.ap()[i * P:(i + 1) * P, :])
                xT.append(t[:, 0:N])
                wqv.append(t[:, N:2 * N])
            wqk = []
            for i in range(CT):
                t = rp.tile([P, 2 * C], BF16, name=f"wqk{i}", tag=f"wqk{i}")
                nc.sync.dma_start(t[:], wqkT_d.ap()[i * P:(i + 1) * P, :])
                wqk.append(t)
            wpj = []
            for i in range(CT):
                t = rp.tile([P, C], BF16, name=f"wpj{i}", tag=f"wpj{i}")
                nc.sync.dma_start(t[:], wprojT_d.ap()[i * P:(i + 1) * P, :])
                wpj.append(t)
            bias_t = rp.tile([P, CT], F32, name="bias_t", tag="bias")
            nc.sync.dma_start(bias_t[:], bias_d.ap())
            ident_t = rp.tile([P, P], BF16, name="ident_t", tag="ident")
            nc.sync.dma_start(ident_t[:], ident_d.ap())

            # ---------------- result tiles ----------------
            qT = [rp.tile([P, N], BF16, name=f"qT{i}", tag=f"qT{i}")
                  for i in range(PAIRS)]
            kT = [rp.tile([P, N], BF16, name=f"kT{i}", tag=f"kT{i}")
                  for i in range(PAIRS)]
            # vt[m]: [128 m-rows, 16 heads, 64 v-dims + ones col]
            vt = [rp.tile([P, H, D + 1], BF16, name=f"vt{m}", tag=f"vt{m}")
                  for m in range(MT)]
            ao = [rp.tile([P, N], BF16, name=f"ao{i}", tag=f"ao{i}")
                  for i in range(PAIRS)]

            for m in range(MT):
                nc.vector.memset(vt[m][:, :, D:D + 1], 1.0)

            # ---------------- emission helpers ----------------
            def vproj_m(m):
                """v-projection for m-tile m: [128 tokens, 1024 v-dims]."""
                ps = pp.tile([P, 1024], F32, name=f"vps{m}", tag="st", bufs=2)
                for j in range(2):
                    for c in range(CT):
                        nc.tensor.matmul(
                            ps[:, j * 512:(j + 1) * 512],
                            xT[c][:, m * P:(m + 1) * P],
                            wqv[c][:, j * 512:(j + 1) * 512],
                            start=(c == 0), stop=(c == CT - 1),
                        )
                nc.vector.tensor_copy(
                    vt[m][:, :, 0:D],
                    ps[:].rearrange("p (h d) -> p h d", d=D),
                )

            def qk_chain_mms(pr, which, n2, tag):
                """Returns (list of mm closures, finish closure)."""
                o0 = which * C + pr * P
                nsl = slice(n2 * 512, (n2 + 1) * 512)
                ps = pp.tile([P, 512], F32, name=f"qk{pr}_{which}_{n2}",
                             tag=tag, bufs=1)
                dst = (qT if which == 0 else kT)[pr]

                def mk(c):
                    def go():
                        nc.tensor.matmul(
                            ps[:],
                            wqk[c][:, o0:o0 + P],
                            xT[c][:, nsl],
                            start=(c == 0), stop=(c == CT - 1),
                        )
                    return go

                def fin():
                    nc.vector.tensor_copy(dst[:, nsl], ps[:])

                return [mk(c) for c in range(CT)], fin

            def s_pair(pr, n2, m):
                """Score matmuls for both heads of the pair + combined exp.
                Returns the pt tile."""
                nsl = slice(n2 * 512, (n2 + 1) * 512)
                msl = slice(m * P, (m + 1) * P)
                st_t = pp.tile([P, 1024], F32, name=f"st{pr}_{n2}_{m}",
                               tag="st", bufs=2)
                for h in range(2):
                    psl = slice(h * 64, (h + 1) * 64)
                    nc.tensor.matmul(
                        st_t[:, h * 512:(h + 1) * 512],
                        kT[pr][psl, msl],
                        qT[pr][psl, nsl],
                        start=True, stop=True,
                        tile_position=(h * 64, 0),
                    )
                pt_t = wp.tile([P, 1024], BF16, name=f"pt{pr}_{n2}_{m}",
                               tag="pt", bufs=18)
                nc.scalar.activation(pt_t[:], st_t[:], AF.Exp, scale=SCALE)
                return pt_t

            def av_chain(pr, n2, h, nu, pts, an_t):
                """Flipped AV for one head and one 128-col n-tile."""
                head = 2 * pr + h
                av_t = pp.tile([P, D + 1], F32, name=f"av{pr}_{n2}_{h}_{nu}",
                               tag="av", bufs=2)
                lo = h * 512 + nu * 128
                for mi in range(MT):
                    nc.tensor.matmul(
                        av_t[:],
                        pts[mi][:, lo:lo + 128],
                        vt[mi][:, head, :],
                        start=(mi == 0), stop=(mi == MT - 1),
                    )
                # normalize + evacuate: per-partition scale by 1/sums col
                rec = wp.tile([P, 1], F32, name=f"rc{pr}_{n2}_{h}_{nu}",
                              tag="rec", bufs=4)
                nc.vector.reciprocal(rec[:], av_t[:, D:D + 1])
                nc.vector.tensor_scalar_mul(
                    an_t[:, h * 64:(h + 1) * 64], av_t[:, 0:D], rec[:])

            partials = {}    # (n2, ot) -> bf16 partial (pr 0..3 sum + bias)

            def proj_lo_chain(n2, ot):
                """First-half output projection (pr 0..3) with bias folded
                in, parked to SBUF; runs in the late ACT-gated slots."""
                ps = pp.tile([P, 512], F32, name=f"ylo{n2}_{ot}", tag="acc",
                             bufs=1)
                nsl = slice(n2 * 512, (n2 + 1) * 512)
                for pr in range(4):
                    nc.tensor.matmul(
                        ps[:],
                        wpj[pr][:, ot * P:(ot + 1) * P],
                        ao[pr][:, nsl],
                        start=(pr == 0), stop=(pr == 3),
                    )
                pt_ = wp.tile([P, 512], BF16, name=f"ypart{n2}_{ot}",
                              tag="part", bufs=8)
                nc.vector.tensor_scalar_add(pt_[:], ps[:],
                                            bias_t[:, ot:ot + 1])
                partials[(n2, ot)] = pt_

            def proj_hi_chain(n2, ot, ps):
                """Second half (pr 4..7) + partial add, for parked tiles."""
                nsl = slice(n2 * 512, (n2 + 1) * 512)
                for pr in range(4, PAIRS):
                    nc.tensor.matmul(
                        ps[:],
                        wpj[pr][:, ot * P:(ot + 1) * P],
                        ao[pr][:, nsl],
                        start=(pr == 4), stop=(pr == PAIRS - 1),
                    )

                def fin():
                    yt = wp.tile([P, 512], BF16, name=f"yh{ot}_{n2}",
                                 tag="y", bufs=3)
                    nc.vector.tensor_add(yt[:], ps[:],
                                         partials[(n2, ot)][:])
                    nc.sync.dma_start(outT_d.ap()[ot * P:(ot + 1) * P, nsl],
                                      yt[:])

                return fin

            def proj_chain_mms(n2, ot, ps):
                """Output projection chain closures for tile (n2, ot)."""
                nsl = slice(n2 * 512, (n2 + 1) * 512)

                def mk(pr):
                    def go():
                        nc.tensor.matmul(
                            ps[:],
                            wpj[pr][:, ot * P:(ot + 1) * P],
                            ao[pr][:, nsl],
                            start=(pr == 0), stop=(pr == PAIRS - 1),
                        )
                    return go

                def fin():
                    yt = wp.tile([P, 512], BF16, name=f"y{ot}_{n2}", tag="y",
                                 bufs=3)
                    nc.vector.tensor_scalar_add(yt[:], ps[:],
                                                bias_t[:, ot:ot + 1])
                    nc.sync.dma_start(outT_d.ap()[ot * P:(ot + 1) * P, nsl],
                                      yt[:])

                return [mk(pr) for pr in range(PAIRS)], fin

            # ---------------- startup ----------------
            # Phase 1: vproj m0..m3 c-OUTER with 8 simultaneously-open psum
            # chains (all 8 banks) so each arriving xw c-tile feeds 8 matmuls
            # immediately -- the PE tracks the DMA stream instead of stalling
            # for the full 4MB xw tensor.
            ps01 = [pp.tile([P, 1024], F32, name=f"vps{m}", tag="st", bufs=2)
                    for m in range(2)]
            ps23 = {(2, 0): pp.tile([P, 512], F32, name="vp2a", tag="acc",
                                    bufs=1),
                    (2, 1): pp.tile([P, 512], F32, name="vp2b", tag="tr",
                                    bufs=1),
                    (3, 0): pp.tile([P, 512], F32, name="vp3a", tag="av",
                                    bufs=2),
                    (3, 1): pp.tile([P, 512], F32, name="vp3b", tag="av",
                                    bufs=2)}
            for c in range(CT):
                for m in range(4):
                    for j in range(2):
                        dst = (ps01[m][:, j * 512:(j + 1) * 512] if m < 2
                               else ps23[(m, j)][:])
                        nc.tensor.matmul(
                            dst,
                            xT[c][:, m * P:(m + 1) * P],
                            wqv[c][:, j * 512:(j + 1) * 512],
                            start=(c == 0), stop=(c == CT - 1),
                        )
            for m in range(2):
                nc.vector.tensor_copy(
                    vt[m][:, :, 0:D],
                    ps01[m][:].rearrange("p (h d) -> p h d", d=D))
            for m in (2, 3):
                for j in range(2):
                    nc.vector.tensor_copy(
                        vt[m][:, j * 8:(j + 1) * 8, 0:D],
                        ps23[(m, j)][:].rearrange("p (h d) -> p h d", d=D))

            # Phase 2: vproj m4..m7 as m-chains, then the pair-0 qk chains
            # (their wqk DMAs land only after all of xw).
            for m in range(4, MT):
                vproj_m(m)
            for which, n2, tag in ((0, 0, "acc"), (0, 1, "tr"),
                                   (1, 0, "acc"), (1, 1, "tr")):
                mms, fin = qk_chain_mms(0, which, n2, tag)
                for go in mms:
                    go()
                fin()

            # ---------------- main loop ----------------
            # Slot (pr, n2).  Filler work per slot:
            #  - previous slot's AV chains + divides + transposes + ao evac
            #  - next pair's q (n2=0 slot) / k (n2=1 slot) projection chains
            #  - vproj m6/m7 in slot (0,0); output projection at pr=7
            pts_prev = None      # (pr, n2, [pt tiles]) of previous slot

            for pr in range(PAIRS):
                for n2 in range(NT2):
                    slot = 2 * pr + n2
                    # --- gather filler: qk chains of next pair ---
                    qk_fill = []
                    if pr < PAIRS - 1:
                        which = n2          # q chains in n2=0, k in n2=1
                        qk_fill.append(
                            qk_chain_mms(pr + 1, which, 0, "acc"))
                        qk_fill.append(
                            qk_chain_mms(pr + 1, which, 1, "tr"))

                    # --- previous slot's AV work ---
                    if pts_prev is not None:
                        ppr, pn2, ppts = pts_prev
                        an_ts = [wp.tile([P, P], BF16,
                                         name=f"an{ppr}_{pn2}_{nu}",
                                         tag="an", bufs=6)
                                 for nu in range(4)]
                        tr_t = pp.tile([P, 512], BF16,
                                       name=f"tr{ppr}_{pn2}", tag="tr",
                                       bufs=1)

                        def mk_av(nu, h, _ppr=ppr, _pn2=pn2, _ppts=ppts,
                                  _an=an_ts):
                            def go():
                                av_chain(_ppr, _pn2, h, nu, _ppts, _an[nu])
                            return go

                        def mk_tr(nu, _an=an_ts, _tr=tr_t):
                            def go():
                                nc.tensor.transpose(
                                    _tr[:, nu * 128:(nu + 1) * 128],
                                    _an[nu][:], ident_t[:])
                            return go

                        def mk_evac(_ppr=ppr, _pn2=pn2, _tr=tr_t):
                            def go():
                                nc.vector.tensor_copy(
                                    ao[_ppr][:, _pn2 * 512:(_pn2 + 1) * 512],
                                    _tr[:])
                            return go

                        av_items = [mk_av(nu, h)
                                    for nu in range(4) for h in range(2)]
                        tr_items = [mk_tr(nu) for nu in range(4)]
                        evac_item = mk_evac()
                    else:
                        av_items, tr_items, evac_item = [], [], None

                    # --- weave the slot ---
                    # filler queue: list of closure-lists, consumed in order
                    # across the 8 m-steps.
                    fq = []
                    if pr == PAIRS - 1:
                        # ACT-gated last slots: fill with partial outproj
                        los = ([(0, 2), (0, 3), (0, 4), (0, 5)] if n2 == 0
                               else [(0, 6), (1, 0), (1, 1), (1, 2)])
                        for lo_n2, lo_ot in los:
                            fq.append([lambda a=lo_n2, b=lo_ot:
                                       proj_lo_chain(a, b)])
                    for mms, fin in qk_fill:
                        def qk_part(items):
                            def go():
                                for it in items:
                                    it()
                            return go
                        fq.append([qk_part(mms[0:4])])
                        fin_ = fin

                        def qk_rest(items=mms[4:8], f=fin_):
                            def go():
                                for it in items:
                                    it()
                                f()
                            return go
                        fq.append([qk_rest()])
                    # AV chains spread over mid/late m-steps, transposes after
                    av_sched = {3: av_items[0:2], 4: av_items[2:4],
                                5: av_items[4:6], 6: av_items[6:8]}
                    tr_sched = {5: tr_items[0:1], 6: tr_items[1:2],
                                7: tr_items[2:4]}

                    pts_now = []
                    for m in range(MT):
                        pts_now.append(s_pair(pr, n2, m))
                        if m < len(fq):
                            for it in fq[m]:
                                it()
                        for it in av_sched.get(m, []):
                            it()
                        for it in tr_sched.get(m, []):
                            it()
                        if m == MT - 1:
                            # leftover filler (slots with >8 filler groups)
                            for grp in fq[MT:]:
                                for it in grp:
                                    it()
                            if evac_item is not None:
                                evac_item()
                    pts_prev = (pr, n2, pts_now)

            # ---------------- tail ----------------
            # last slot's AV + transposes, then the rest of the projection
            ppr, pn2, ppts = pts_prev
            an_ts = [wp.tile([P, P], BF16, name=f"an{ppr}_{pn2}_{nu}",
                             tag="an", bufs=6) for nu in range(4)]
            tr_t = pp.tile([P, 512], BF16, name=f"tr{ppr}_{pn2}", tag="tr",
                           bufs=1)

            # fill the E(7,1,7) wait with two n2=0 proj chains on st halves
            st_tail = pp.tile([P, 1024], F32, name="st_tail", tag="st",
                              bufs=2)
            tail_fins = []
            for j, ot in enumerate((0, 1)):
                mms, fin = proj_chain_mms(0, ot,
                                          st_tail[:, j * 512:(j + 1) * 512])
                for go in mms[0:4]:
                    go()
                tail_fins.append((mms[4:], fin))

            for nu in range(4):
                for h in range(2):
                    av_chain(ppr, pn2, h, nu, ppts, an_ts[nu])
                if nu >= 1 and tail_fins:
                    mms, fin = tail_fins.pop(0)
                    for go in mms:
                        go()
                    fin()
            for mms, fin in tail_fins:
                for go in mms:
                    go()
                fin()
            for nu in range(4):
                nc.tensor.transpose(tr_t[:, nu * 128:(nu + 1) * 128],
                                    an_ts[nu][:], ident_t[:])
            nc.vector.tensor_copy(ao[ppr][:, pn2 * 512:(pn2 + 1) * 512],
                                  tr_t[:])

            # remaining projection: n2=0 tiles first (ao[7] n2=0 is ready a
            # slot earlier than n2=1); parked n2=1 hi-chains last -- the
            # final chain is short (4 matmuls) to minimize the drain tail.
            remaining = [(0, ot) for ot in range(2, CT)]
            remaining += [(1, ot) for ot in range(CT)]
            remaining.sort(key=lambda t: (t not in partials, t))
            # the short parked (1,1) goes last to minimize the drain tail
            remaining.remove((1, 1))
            remaining.append((1, 1))
            tags = ["st2", "st2", "st3", "st3", "acc", "tr", "av", "av"]
            st2 = pp.tile([P, 1024], F32, name="st2", tag="st", bufs=2)
            st3 = pp.tile([P, 1024], F32, name="st3", tag="st", bufs=2)
            fins = []
            for i, (n2, ot) in enumerate(remaining):
                tg = tags[i % 8]
                if tg == "st2":
                    ps = st2[:, (i % 2) * 512:((i % 2) + 1) * 512]
                elif tg == "st3":
                    ps = st3[:, (i % 2) * 512:((i % 2) + 1) * 512]
                elif tg == "av":
                    ps = pp.tile([P, 512], F32, name=f"ytail{i}", tag="av",
                                 bufs=2)
                else:
                    ps = pp.tile([P, 512], F32, name=f"ytail{i}", tag=tg,
                                 bufs=1)
                last = (i == len(remaining) - 1)
                if last and (n2, ot) in partials:
                    # split the final parked tile into halves (separate psum
                    # banks) so the first half's add/DMA overlaps the second
                    # half's matmuls
                    ps2 = pp.tile([P, 256], F32, name="yfin_ps2", tag="av",
                                  bufs=2)
                    for half in range(2):
                        pst = ps if half == 0 else ps2
                        csl = slice(half * 256, (half + 1) * 256)
                        asl = slice(n2 * 512 + half * 256,
                                    n2 * 512 + (half + 1) * 256)
                        for pr in range(4, PAIRS):
                            nc.tensor.matmul(
                                pst[:, 0:256],
                                wpj[pr][:, ot * P:(ot + 1) * P],
                                ao[pr][:, asl],
                                start=(pr == 4), stop=(pr == PAIRS - 1),
                            )
                        yt = wp.tile([P, 256], BF16, name=f"yfin{half}",
                                     tag="y", bufs=3)
                        nc.vector.tensor_add(yt[:], pst[:, 0:256],
                                             partials[(n2, ot)][:, csl])
                        nc.sync.dma_start(
                            outT_d.ap()[ot * P:(ot + 1) * P, asl], yt[:])
                        if half == 0:
                            for fin in fins:
                                fin()
                            fins = []
                elif (n2, ot) in partials:
                    fins.append(proj_hi_chain(n2, ot, ps))
                    if len(fins) >= 2:
                        fins.pop(0)()
                elif not last:
                    mms, fin = proj_chain_mms(n2, ot, ps)
                    for go in mms:
                        go()
                    fins.append(fin)
                    # drain finishes with one-chain delay so bufs recycle
                    if len(fins) >= 2:
                        fins.pop(0)()
                else:
                    # split the last output tile into halves (separate psum
                    # banks -- a shared bank would serialize half1's start
                    # behind half0's bias-add read via the zero region) so
                    # the first bias-add/DMA overlaps the second half's mms
                    ps2 = pp.tile([P, 256], F32, name="ylast_ps2", tag="av",
                                  bufs=2)
                    for half in range(2):
                        pst = ps if half == 0 else ps2
                        for pr in range(PAIRS):
                            nc.tensor.matmul(
                                pst[:, 0:256],
                                wpj[pr][:, ot * P:(ot + 1) * P],
                                ao[pr][:, n2 * 512 + half * 256:
                                       n2 * 512 + (half + 1) * 256],
                                start=(pr == 0), stop=(pr == PAIRS - 1),
                            )
                        yt = wp.tile([P, 256], BF16, name=f"ylast{half}",
                                     tag="y", bufs=3)
                        nc.vector.tensor_scalar_add(yt[:], pst[:, 0:256],
                                                    bias_t[:, ot:ot + 1])
                        nc.sync.dma_start(
                            outT_d.ap()[ot * P:(ot + 1) * P,
                                        n2 * 512 + half * 256:
                                        n2 * 512 + (half + 1) * 256],
                            yt[:])
                        if half == 0:
                            for fin in fins:
                                fin()
                            fins = []
            for fin in fins:
                fin()

    nc.compile()
    return nc


def get_nc():
    if "nc" not in _cache:
        _cache["nc"] = _build()
    return _cache["nc"]


def kernel(x, w_qkv, w_proj, b_proj):
    x = np.asarray(x, dtype=np.float32)
    w_qkv = np.asarray(w_qkv, dtype=np.float32)
    w_proj = np.asarray(w_proj, dtype=np.float32)
    b_proj = np.asarray(b_proj, dtype=np.float32)

    bf = ml_dtypes.bfloat16
    wqkvT = np.ascontiguousarray(w_qkv.T).astype(bf)     # [C, 3C]
    wqkT = np.ascontiguousarray(wqkvT[:, 0:2 * C])       # [C, 2C] q,k cols
    wprojT = np.ascontiguousarray(w_proj.T).astype(bf)   # [C, C]
    bias = np.ascontiguousarray(b_proj.reshape(CT, P).T).astype(np.float32)
    ident = np.eye(P, dtype=bf)

    in_maps = []
    wqv_host = wqkvT[:, 2 * C:]                          # [C, C] v columns
    for b in range(N_CORES):
        xT = np.ascontiguousarray(x[b].T).astype(bf)     # [C, N]
        xw = np.ascontiguousarray(np.concatenate([xT, wqv_host], axis=1))
        in_maps.append({"xw": xw, "wqkT": wqkT, "wprojT": wprojT,
                        "bias": bias, "ident": ident})

    nc = get_nc()
    _cache["in_maps"] = in_maps
    res = bass_utils.run_bass_kernel_spmd(nc, in_maps,
                                          core_ids=list(range(N_CORES)))
    out = np.empty((B, N, C), dtype=np.float32)
    for b in range(N_CORES):
        out[b] = res.results[b]["outT"].T.astype(np.float32)
    return out


# revision 30
# speedup vs baseline: 1.0041x; 1.0041x over previous
"""Trainium2 Bass kernel for batched multi-head self-attention.

Reference computation (per batch element b):
    qkv = x @ w_qkv.T                  # [N, 3C]
    q, k, v = split/reshape to heads   # H=16 heads, d=64
    attn = softmax(q @ k.T / sqrt(d))
    out = (attn @ v) reshaped back     # [N, C]
    y = out @ w_proj.T + b_proj

Sharding: pure data-parallel over batch B=8 across the 8 NeuronCores
(one batch element per core, weights replicated, no collectives).

On-device layout (everything transposed so matmuls contract over the
partition axis with no input transposes):
  - xT      [C, N]   (host pre-transposed, bf16)
  - wqkT    [C, 2C]  (host pre-transposed q,k columns, bf16)
  - wprojT  [C, C]   (host pre-transposed, bf16)

Cost model: a PE matmul costs out_free_size * 0.4167ns regardless of
contraction size or output partitions.  The key restructure vs the
229us baseline is the attn@V matmul orientation:
  - scores S^T [m,n] tiles (K=64, head pairs row-packed): free=512.
  - AV is computed FLIPPED: out[n, d+1] = pt[m, n-tile].T @ [V | 1]
    with free=65 instead of the old [d+1, 512] free=512 layout.
    This halves AV PE time (54.6us -> 27.7us).  The ones column gives
    softmax row sums per output partition, so normalization is a
    per-partition tensor_scalar divide on DVE (no partition
    broadcasts).
  - The [n, c] result is transposed back to [c, n] for the output
    projection with PE transpose instructions (128x128 via identity,
    53ns each, 64 total = 3.4us).
  - exp() is done on [128, 1024] head-pair tiles (one ACT instruction
    per (n2, m)) to amortize the ~185ns ACT access-latency overhead;
    ACT total 133us stays under the PE's ~197us.
Emission interleaves each attention slot (pr, n2) with the previous
slot's AV/transpose work and the next pair's q/k projection chains so
the PE never waits on ACT; the output projection fills the tail.
PSUM budget (8 banks): st 2x2 + acc 1 + av 2 + tr 1.
"""

import os
import sys

for _p in ("/opt/trn_rl_repo", "/root/.axon_site/_ro/trn_rl_repo"):
    if os.path.isdir(_p) and _p not in sys.path:
        sys.path.insert(0, _p)
        break

import numpy as np
import ml_dtypes

import concourse.bass as bass
import concourse.bacc as bacc
import concourse.tile as tile
import concourse.mybir as mybir
from concourse import bass_utils

BF16 = mybir.dt.bfloat16
F32 = mybir.dt.float32
AF = mybir.ActivationFunctionType
ALU = mybir.AluOpType

B, N, C, H = 8, 1024, 1024, 16
D = C // H            # 64 head dim
P = 128               # partitions
CT = C // P           # 8 contraction tiles
NT2 = N // 512        # 2 n-tiles of 512
MT = N // P           # 8 m-tiles of 128
PAIRS = H // 2        # 8 head pairs
SCALE = float(D) ** -0.5
N_CORES = 8

_cache = {}


def _build():
    nc = bacc.Bacc("TRN2", target_bir_lowering=False, debug=False,
                   enable_asserts=False, num_devices=N_CORES)

    xw_d = nc.dram_tensor("xw", [C, 2 * N], BF16, kind="ExternalInput")
    wqkT_d = nc.dram_tensor("wqkT", [C, 2 * C], BF16, kind="ExternalInput")
    wprojT_d = nc.dram_tensor("wprojT", [C, C], BF16, kind="ExternalInput")
    bias_d = nc.dram_tensor("bias", [P, CT], F32, kind="ExternalInput")
    ident_d = nc.dram_tensor("ident", [P, P], BF16, kind="ExternalInput")
    outT_d = nc.dram_tensor("outT", [C, N], BF16, kind="ExternalOutput")

    with tile.TileContext(nc) as tc:
        with (
            tc.tile_pool(name="res", bufs=1) as rp,
            tc.tile_pool(name="work", bufs=2) as wp,
            tc.tile_pool(name="ps", bufs=1, space="PSUM") as pp,
        ):
            # ---------------- PE warm-up ----------------
            # Cover the initial input-DMA wait with dummy matmuls so the
            # p-state/HAM ramp completes before real work arrives.
            warm_a = wp.tile([P, 512], BF16, name="warm_a", tag="warm_a",
                             bufs=1)
            nc.vector.memset(warm_a[:], 0.25)
            warm_ps = pp.tile([P, 512], F32, name="warm_ps", tag="acc",
                              bufs=1)
            for _ in range(5):
                nc.tensor.matmul(warm_ps[:], warm_a[:, 0:P], warm_a[:],
                                 start=True, stop=True)

            # ---------------- resident inputs ----------------
            xT = []
            wqv = []
            for i in range(CT):
                t = rp.tile([P, 2 * N], BF16, name=f"xw{i}", tag=f"xw{i}")
                nc.sync.dma_start(t[:], xw_d.ap()[i * P:(i + 1) * P, :])
                xT.append(t[:, 0:N])
                wqv.append(t[:, N:2 * N])
            wqk = []
            for i in range(CT):
                t = rp.tile([P, 2 * C], BF16, name=f"wqk{i}", tag=f"wqk{i}")
                nc.sync.dma_start(t[:], wqkT_d.ap()[i * P:(i + 1) * P, :])
                wqk.append(t)
            wpj = []
            for i in range(CT):
                t = rp.tile([P, C], BF16, name=f"wpj{i}", tag=f"wpj{i}")
                nc.sync.dma_start(t[:], wprojT_d.ap()[i * P:(i + 1) * P, :])
                wpj.append(t)
            bias_t = rp.tile([P, CT], F32, name="bias_t", tag="bias")
            nc.sync.dma_start(bias_t[:], bias_d.ap())
            ident_t = rp.tile([P, P], BF16, name="ident_t", tag="ident")
            nc.sync.dma_start(ident_t[:], ident_d.ap())

            # ---------------- result tiles ----------------
            qT = [rp.tile([P, N], BF16, name=f"qT{i}", tag=f"qT{i}")
                  for i in range(PAIRS)]
            kT = [rp.tile([P, N], BF16, name=f"kT{i}", tag=f"kT{i}")
                  for i in range(PAIRS)]
            # vt[m]: [128 m-rows, 16 heads, 64 v-dims + ones col]
            vt = [rp.tile([P, H, D + 1], BF16, name=f"vt{m}", tag=f"vt{m}")
                  for m in range(MT)]
            ao = [rp.tile([P, N], BF16, name=f"ao{i}", tag=f"ao{i}")
                  for i in range(PAIRS)]

            for m in range(MT):
                nc.vector.memset(vt[m][:, :, D:D + 1], 1.0)

            # ---------------- emission helpers ----------------
            def vproj_m(m):
                """v-projection for m-tile m: [128 tokens, 1024 v-dims]."""
                ps = pp.tile([P, 1024], F32, name=f"vps{m}", tag="st", bufs=2)
                for j in range(2):
                    for c in range(CT):
                        nc.tensor.matmul(
                            ps[:, j * 512:(j + 1) * 512],
                            xT[c][:, m * P:(m + 1) * P],
                            wqv[c][:, j * 512:(j + 1) * 512],
                            start=(c == 0), stop=(c == CT - 1),
                        )
                nc.vector.tensor_copy(
                    vt[m][:, :, 0:D],
                    ps[:].rearrange("p (h d) -> p h d", d=D),
                )

            def qk_chain_mms(pr, which, n2, tag):
                """Returns (list of mm closures, finish closure)."""
                o0 = which * C + pr * P
                nsl = slice(n2 * 512, (n2 + 1) * 512)
                ps = pp.tile([P, 512], F32, name=f"qk{pr}_{which}_{n2}",
                             tag=tag, bufs=1)
                dst = (qT if which == 0 else kT)[pr]

                def mk(c):
                    def go():
                        nc.tensor.matmul(
                            ps[:],
                            wqk[c][:, o0:o0 + P],
                            xT[c][:, nsl],
                            start=(c == 0), stop=(c == CT - 1),
                        )
                    return go

                def fin():
                    nc.vector.tensor_copy(dst[:, nsl], ps[:])

                return [mk(c) for c in range(CT)], fin

            def s_pair(pr, n2, m):
                """Score matmuls for both heads of the pair + combined exp.
                Returns the pt tile."""
                nsl = slice(n2 * 512, (n2 + 1) * 512)
                msl = slice(m * P, (m + 1) * P)
                st_t = pp.tile([P, 1024], F32, name=f"st{pr}_{n2}_{m}",
                               tag="st", bufs=2)
                for h in range(2):
                    psl = slice(h * 64, (h + 1) * 64)
                    nc.tensor.matmul(
                        st_t[:, h * 512:(h + 1) * 512],
                        kT[pr][psl, msl],
                        qT[pr][psl, nsl],
                        start=True, stop=True,
                        tile_position=(h * 64, 0),
                    )
                pt_t = wp.tile([P, 1024], BF16, name=f"pt{pr}_{n2}_{m}",
                               tag="pt", bufs=18)
                nc.scalar.activation(pt_t[:], st_t[:], AF.Exp, scale=SCALE)
                return pt_t

            def av_chain(pr, n2, h, nu, pts, an_t):
                """Flipped AV for one head and one 128-col n-tile."""
                head = 2 * pr + h
                av_t = pp.tile([P, D + 1], F32, name=f"av{pr}_{n2}_{h}_{nu}",
                               tag="av", bufs=2)
                lo = h * 512 + nu * 128
                for mi in range(MT):
                    nc.tensor.matmul(
                        av_t[:],
                        pts[mi][:, lo:lo + 128],
                        vt[mi][:, head, :],
                        start=(mi == 0), stop=(mi == MT - 1),
                    )
                # normalize + evacuate: per-partition scale by 1/sums col
                rec = wp.tile([P, 1], F32, name=f"rc{pr}_{n2}_{h}_{nu}",
                              tag="rec", bufs=4)
                nc.vector.reciprocal(rec[:], av_t[:, D:D + 1])
                nc.vector.tensor_scalar_mul(
                    an_t[:, h * 64:(h + 1) * 64], av_t[:, 0:D], rec[:])

            partials = {}    # (n2, ot) -> bf16 partial (pr 0..3 sum + bias)

            def proj_lo_chain(n2, ot):
                """First-half output projection (pr 0..3) with bias folded
                in, parked to SBUF; runs in the late ACT-gated slots."""
                ps = pp.tile([P, 512], F32, name=f"ylo{n2}_{ot}", tag="acc",
                             bufs=1)
                nsl = slice(n2 * 512, (n2 + 1) * 512)
                for pr in range(4):
                    nc.tensor.matmul(
                        ps[:],
                        wpj[pr][:, ot * P:(ot + 1) * P],
                        ao[pr][:, nsl],
                        start=(pr == 0), stop=(pr == 3),
                    )
                pt_ = wp.tile([P, 512], BF16, name=f"ypart{n2}_{ot}",
                              tag="part", bufs=8)
                nc.vector.tensor_scalar_add(pt_[:], ps[:],
                                            bias_t[:, ot:ot + 1])
                partials[(n2, ot)] = pt_

            def proj_hi_chain(n2, ot, ps):
                """Second half (pr 4..7) + partial add, for parked tiles."""
                nsl = slice(n2 * 512, (n2 + 1) * 512)
                for pr in range(4, PAIRS):
                    nc.tensor.matmul(
                        ps[:],
                        wpj[pr][:, ot * P:(ot + 1) * P],
                        ao[pr][:, nsl],
                        start=(pr == 4), stop=(pr == PAIRS - 1),
                    )

                def fin():
                    yt = wp.tile([P, 512], BF16, name=f"yh{ot}_{n2}",
                                 tag="y", bufs=3)
                    nc.vector.tensor_add(yt[:], ps[:],
                                         partials[(n2, ot)][:])
                    nc.sync.dma_start(outT_d.ap()[ot * P:(ot + 1) * P, nsl],
                                      yt[:])

                return fin

            def proj_chain_mms(n2, ot, ps):
                """Output projection chain closures for tile (n2, ot)."""
                nsl = slice(n2 * 512, (n2 + 1) * 512)

                def mk(pr):
                    def go():
                        nc.tensor.matmul(
                            ps[:],
                            wpj[pr][:, ot * P:(ot + 1) * P],
                            ao[pr][:, nsl],
                            start=(pr == 0), stop=(pr == PAIRS - 1),
                        )
                    return go

                def fin():
                    yt = wp.tile([P, 512], BF16, name=f"y{ot}_{n2}", tag="y",
                                 bufs=3)
                    nc.vector.tensor_scalar_add(yt[:], ps[:],
                                                bias_t[:, ot:ot + 1])
                    nc.sync.dma_start(outT_d.ap()[ot * P:(ot + 1) * P, nsl],
                                      yt[:])

                return [mk(pr) for pr in range(PAIRS)], fin

            # ---------------- startup ----------------
            # Phase 1: vproj m0..m3 c-OUTER with 8 simultaneously-open psum
            # chains (all 8 banks) so each arriving xw c-tile feeds 8 matmuls
            # immediately -- the PE tracks the DMA stream instead of stalling
            # for the full 4MB xw tensor.
            ps01 = [pp.tile([P, 1024], F32, name=f"vps{m}", tag="st", bufs=2)
                    for m in range(2)]
            ps23 = {(2, 0): pp.tile([P, 512], F32, name="vp2a", tag="acc",
                                    bufs=1),
                    (2, 1): pp.tile([P, 512], F32, name="vp2b", tag="tr",
                                    bufs=1),
                    (3, 0): pp.tile([P, 512], F32, name="vp3a", tag="av",
                                    bufs=2),
                    (3, 1): pp.tile([P, 512], F32, name="vp3b", tag="av",
                                    bufs=2)}
            for c in range(CT):
                for m in range(4):
                    for j in range(2):
                        dst = (ps01[m][:, j * 512:(j + 1) * 512] if m < 2
                               else ps23[(m, j)][:])
                        nc.tensor.matmul(
                            dst,
                            xT[c][:, m * P:(m + 1) * P],
                            wqv[c][:, j * 512:(j + 1) * 512],
                            start=(c == 0), stop=(c == CT - 1),
                        )
            for m in range(2):
                nc.vector.tensor_copy(
                    vt[m][:, :, 0:D],
                    ps01[m][:].rearrange("p (h d) -> p h d", d=D))
            for m in (2, 3):
                for j in range(2):
                    nc.vector.tensor_copy(
                        vt[m][:, j * 8:(j + 1) * 8, 0:D],
                        ps23[(m, j)][:].rearrange("p (h d) -> p h d", d=D))

            # Phase 2: vproj m4..m7 as m-chains, then the pair-0 qk chains
            # (their wqk DMAs land only after all of xw).
            for m in range(4, MT):
                vproj_m(m)
            for which, n2, tag in ((0, 0, "acc"), (0, 1, "tr"),
                                   (1, 0, "acc"), (1, 1, "tr")):
                mms, fin = qk_chain_mms(0, which, n2, tag)
                for go in mms:
                    go()
                fin()

            # ---------------- main loop ----------------
            # Slot (pr, n2).  Filler work per slot:
            #  - previous slot's AV chains + divides + transposes + ao evac
            #  - next pair's q (n2=0 slot) / k (n2=1 slot) projection chains
            #  - vproj m6/m7 in slot (0,0); output projection at pr=7
            pts_prev = None      # (pr, n2, [pt tiles]) of previous slot

            for pr in range(PAIRS):
                for n2 in range(NT2):
                    slot = 2 * pr + n2
                    # --- gather filler: qk chains of next pair ---
                    qk_fill = []
                    if pr < PAIRS - 1:
                        which = n2          # q chains in n2=0, k in n2=1
                        qk_fill.append(
                            qk_chain_mms(pr + 1, which, 0, "acc"))
                        qk_fill.append(
                            qk_chain_mms(pr + 1, which, 1, "tr"))

                    # --- previous slot's AV work ---
                    if pts_prev is not None:
                        ppr, pn2, ppts = pts_prev
                        an_ts = [wp.tile([P, P], BF16,
                                         name=f"an{ppr}_{pn2}_{nu}",
                                         tag="an", bufs=6)
                                 for nu in range(4)]
                        tr_t = pp.tile([P, 512], BF16,
                                       name=f"tr{ppr}_{pn2}", tag="tr",
                                       bufs=1)

                        def mk_av(nu, h, _ppr=ppr, _pn2=pn2, _ppts=ppts,
                                  _an=an_ts):
                            def go():
                                av_chain(_ppr, _pn2, h, nu, _ppts, _an[nu])
                            return go

                        def mk_tr(nu, _an=an_ts, _tr=tr_t):
                            def go():
                                nc.tensor.transpose(
                                    _tr[:, nu * 128:(nu + 1) * 128],
                                    _an[nu][:], ident_t[:])
                            return go

                        def mk_evac(_ppr=ppr, _pn2=pn2, _tr=tr_t):
                            def go():
                                nc.vector.tensor_copy(
                                    ao[_ppr][:, _pn2 * 512:(_pn2 + 1) * 512],
                                    _tr[:])
                            return go

                        av_items = [mk_av(nu, h)
                                    for nu in range(4) for h in range(2)]
                        tr_items = [mk_tr(nu) for nu in range(4)]
                        evac_item = mk_evac()
                    else:
                        av_items, tr_items, evac_item = [], [], None

                    # --- weave the slot ---
                    # filler queue: list of closure-lists, consumed in order
                    # across the 8 m-steps.
                    fq = []
                    if pr == PAIRS - 1:
                        # ACT-gated last slots: fill with partial outproj
                        los = ([(0, 2), (0, 3), (0, 4), (0, 5)] if n2 == 0
                               else [(0, 6), (1, 0), (1, 1), (1, 2)])
                        for lo_n2, lo_ot in los:
                            fq.append([lambda a=lo_n2, b=lo_ot:
                                       proj_lo_chain(a, b)])
                    for mms, fin in qk_fill:
                        def qk_part(items):
                            def go():
                                for it in items:
                                    it()
                            return go
                        fq.append([qk_part(mms[0:4])])
                        fin_ = fin

                        def qk_rest(items=mms[4:8], f=fin_):
                            def go():
                                for it in items:
                                    it()
                                f()
                            return go
                        fq.append([qk_rest()])
                    # AV chains spread over mid/late m-steps, transposes after
                    av_sched = {3: av_items[0:2], 4: av_items[2:4],
                                5: av_items[4:6], 6: av_items[6:8]}
                    tr_sched = {5: tr_items[0:1], 6: tr_items[1:2],
                                7: tr_items[2:4]}

                    pts_now = []
                    for m in range(MT):
                        pts_now.append(s_pair(pr, n2, m))
                        if m < len(fq):
                            for it in fq[m]:
                                it()
                        for it in av_sched.get(m, []):
                            it()
                        for it in tr_sched.get(m, []):
                            it()
                        if m == MT - 1:
                            # leftover filler (slots with >8 filler groups)
                            for grp in fq[MT:]:
                                for it in grp:
                                    it()
                            if evac_item is not None:
                                evac_item()
                    pts_prev = (pr, n2, pts_now)

            # ---------------- tail ----------------
            # last slot's AV + transposes, then the rest of the projection
            ppr, pn2, ppts = pts_prev
            an_ts = [wp.tile([P, P], BF16, name=f"an{ppr}_{pn2}_{nu}",
                             tag="an", bufs=6) for nu in range(4)]
            tr_t = pp.tile([P, 512], BF16, name=f"tr{ppr}_{pn2}", tag="tr",
                           bufs=1)

            # fill the E(7,1,7) wait with two n2=0 proj chains on st halves
            st_tail = pp.tile([P, 1024], F32, name="st_tail", tag="st",
                              bufs=2)
            tail_fins = []
            for j, ot in enumerate((0, 1)):
                mms, fin = proj_chain_mms(0, ot,
                                          st_tail[:, j * 512:(j + 1) * 512])
                for go in mms[0:4]:
                    go()
                tail_fins.append((mms[4:], fin))

            for nu in range(4):
                for h in range(2):
                    av_chain(ppr, pn2, h, nu, ppts, an_ts[nu])
                if nu >= 1 and tail_fins:
                    mms, fin = tail_fins.pop(0)
                    for go in mms:
                        go()
                    fin()
            for mms, fin in tail_fins:
                for go in mms:
                    go()
                fin()
            for nu in range(4):
                nc.tensor.transpose(tr_t[:, nu * 128:(nu + 1) * 128],
                                    an_ts[nu][:], ident_t[:])
            nc.vector.tensor_copy(ao[ppr][:, pn2 * 512:(pn2 + 1) * 512],
                                  tr_t[:])

            # remaining projection: n2=0 tiles first (ao[7] n2=0 is ready a
            # slot earlier than n2=1); parked n2=1 hi-chains last -- the
            # final chain is short (4 matmuls) to minimize the drain tail.
            remaining = [(0, ot) for ot in range(2, CT)]
            remaining += [(1, ot) for ot in range(CT)]
            remaining.sort(key=lambda t: (t not in partials, t))
            tags = ["st2", "st2", "st3", "st3", "acc", "tr", "av", "av"]
            st2 = pp.tile([P, 1024], F32, name="st2", tag="st", bufs=2)
            st3 = pp.tile([P, 1024], F32, name="st3", tag="st", bufs=2)
            fins = []
            for i, (n2, ot) in enumerate(remaining):
                tg = tags[i % 8]
                if tg == "st2":
                    ps = st2[:, (i % 2) * 512:((i % 2) + 1) * 512]
                elif tg == "st3":
                    ps = st3[:, (i % 2) * 512:((i % 2) + 1) * 512]
                elif tg == "av":
                    ps = pp.tile([P, 512], F32, name=f"ytail{i}", tag="av",
                                 bufs=2)
                else:
                    ps = pp.tile([P, 512], F32, name=f"ytail{i}", tag=tg,
                                 bufs=1)
                last = (i == len(remaining) - 1)
                if last and (n2, ot) in partials:
                    # split the final parked tile into halves (separate psum
                    # banks) so the first half's add/DMA overlaps the second
                    # half's matmuls
                    ps2 = pp.tile([P, 256], F32, name="yfin_ps2", tag="av",
                                  bufs=2)
                    for half in range(2):
                        pst = ps if half == 0 else ps2
                        csl = slice(half * 256, (half + 1) * 256)
                        asl = slice(n2 * 512 + half * 256,
                                    n2 * 512 + (half + 1) * 256)
                        for pr in range(4, PAIRS):
                            nc.tensor.matmul(
                                pst[:, 0:256],
                                wpj[pr][:, ot * P:(ot + 1) * P],
                                ao[pr][:, asl],
                                start=(pr == 4), stop=(pr == PAIRS - 1),
                            )
                        yt = wp.tile([P, 256], BF16, name=f"yfin{half}",
                                     tag="y", bufs=3)
                        nc.vector.tensor_add(yt[:], pst[:, 0:256],
                                             partials[(n2, ot)][:, csl])
                        nc.sync.dma_start(
                            outT_d.ap()[ot * P:(ot + 1) * P, asl], yt[:])
                        if half == 0:
                            for fin in fins:
                                fin()
                            fins = []
                elif (n2, ot) in partials:
                    fins.append(proj_hi_chain(n2, ot, ps))
                    if len(fins) >= 2:
                        fins.pop(0)()
                elif not last:
                    mms, fin = proj_chain_mms(n2, ot, ps)
                    for go in mms:
                        go()
                    fins.append(fin)
                    # drain finishes with one-chain delay so bufs recycle
                    if len(fins) >= 2:
                        fins.pop(0)()
                else:
                    # split the last output tile into halves (separate psum
                    # banks -- a shared bank would serialize half1's start
                    # behind half0's bias-add read via the zero region) so
                    # the first bias-add/DMA overlaps the second half's mms
                    ps2 = pp.tile([P, 256], F32, name="ylast_ps2", tag="av",
                                  bufs=2)
                    for half in range(2):
                        pst = ps if half == 0 else ps2
                        for pr in range(PAIRS):
                            nc.tensor.matmul(
                                pst[:, 0:256],
                                wpj[pr][:, ot * P:(ot + 1) * P],
                                ao[pr][:, n2 * 512 + half * 256:
                                       n2 * 512 + (half + 1) * 256],
                                start=(pr == 0), stop=(pr == PAIRS - 1),
                            )
                        yt = wp.tile([P, 256], BF16, name=f"ylast{half}",
                                     tag="y", bufs=3)
                        nc.vector.tensor_scalar_add(yt[:], pst[:, 0:256],
                                                    bias_t[:, ot:ot + 1])
                        nc.sync.dma_start(
                            outT_d.ap()[ot * P:(ot + 1) * P,
                                        n2 * 512 + half * 256:
                                        n2 * 512 + (half + 1) * 256],
                            yt[:])
                        if half == 0:
                            for fin in fins:
                                fin()
                            fins = []
            for fin in fins:
                fin()

    nc.compile()
    return nc


def get_nc():
    if "nc" not in _cache:
        _cache["nc"] = _build()
    return _cache["nc"]


def kernel(x, w_qkv, w_proj, b_proj):
    x = np.asarray(x, dtype=np.float32)
    w_qkv = np.asarray(w_qkv, dtype=np.float32)
    w_proj = np.asarray(w_proj, dtype=np.float32)
    b_proj = np.asarray(b_proj, dtype=np.float32)

    bf = ml_dtypes.bfloat16
    wqkvT = np.ascontiguousarray(w_qkv.T).astype(bf)     # [C, 3C]
    wqkT = np.ascontiguousarray(wqkvT[:, 0:2 * C])       # [C, 2C] q,k cols
    wprojT = np.ascontiguousarray(w_proj.T).astype(bf)   # [C, C]
    bias = np.ascontiguousarray(b_proj.reshape(CT, P).T).astype(np.float32)
    ident = np.eye(P, dtype=bf)

    in_maps = []
    wqv_host = wqkvT[:, 2 * C:]                          # [C, C] v columns
    for b in range(N_CORES):
        xT = np.ascontiguousarray(x[b].T).astype(bf)     # [C, N]
        xw = np.ascontiguousarray(np.concatenate([xT, wqv_host], axis=1))
        in_maps.append({"xw": xw, "wqkT": wqkT, "wprojT": wprojT,
                        "bias": bias, "ident": ident})

    nc = get_nc()
    _cache["in_maps"] = in_maps
    res = bass_utils.run_bass_kernel_spmd(nc, in_maps,
                                          core_ids=list(range(N_CORES)))
    out = np.empty((B, N, C), dtype=np.float32)
    for b in range(N_CORES):
        out[b] = res.results[b]["outT"].T.astype(np.float32)
    return out


# revision 31
# speedup vs baseline: 1.0044x; 1.0003x over previous
"""Trainium2 Bass kernel for batched multi-head self-attention.

Reference computation (per batch element b):
    qkv = x @ w_qkv.T                  # [N, 3C]
    q, k, v = split/reshape to heads   # H=16 heads, d=64
    attn = softmax(q @ k.T / sqrt(d))
    out = (attn @ v) reshaped back     # [N, C]
    y = out @ w_proj.T + b_proj

Sharding: pure data-parallel over batch B=8 across the 8 NeuronCores
(one batch element per core, weights replicated, no collectives).

On-device layout (everything transposed so matmuls contract over the
partition axis with no input transposes):
  - xT      [C, N]   (host pre-transposed, bf16)
  - wqkT    [C, 2C]  (host pre-transposed q,k columns, bf16)
  - wprojT  [C, C]   (host pre-transposed, bf16)

Cost model: a PE matmul costs out_free_size * 0.4167ns regardless of
contraction size or output partitions.  The key restructure vs the
229us baseline is the attn@V matmul orientation:
  - scores S^T [m,n] tiles (K=64, head pairs row-packed): free=512.
  - AV is computed FLIPPED: out[n, d+1] = pt[m, n-tile].T @ [V | 1]
    with free=65 instead of the old [d+1, 512] free=512 layout.
    This halves AV PE time (54.6us -> 27.7us).  The ones column gives
    softmax row sums per output partition, so normalization is a
    per-partition tensor_scalar divide on DVE (no partition
    broadcasts).
  - The [n, c] result is transposed back to [c, n] for the output
    projection with PE transpose instructions (128x128 via identity,
    53ns each, 64 total = 3.4us).
  - exp() is done on [128, 1024] head-pair tiles (one ACT instruction
    per (n2, m)) to amortize the ~185ns ACT access-latency overhead;
    ACT total 133us stays under the PE's ~197us.
Emission interleaves each attention slot (pr, n2) with the previous
slot's AV/transpose work and the next pair's q/k projection chains so
the PE never waits on ACT; the output projection fills the tail.
PSUM budget (8 banks): st 2x2 + acc 1 + av 2 + tr 1.
"""

import os
import sys

for _p in ("/opt/trn_rl_repo", "/root/.axon_site/_ro/trn_rl_repo"):
    if os.path.isdir(_p) and _p not in sys.path:
        sys.path.insert(0, _p)
        break

import numpy as np
import ml_dtypes

import concourse.bass as bass
import concourse.bacc as bacc
import concourse.tile as tile
import concourse.mybir as mybir
from concourse import bass_utils

BF16 = mybir.dt.bfloat16
F32 = mybir.dt.float32
AF = mybir.ActivationFunctionType
ALU = mybir.AluOpType

B, N, C, H = 8, 1024, 1024, 16
D = C // H            # 64 head dim
P = 128               # partitions
CT = C // P           # 8 contraction tiles
NT2 = N // 512        # 2 n-tiles of 512
MT = N // P           # 8 m-tiles of 128
PAIRS = H // 2        # 8 head pairs
SCALE = float(D) ** -0.5
N_CORES = 8

_cache = {}


def _build():
    nc = bacc.Bacc("TRN2", target_bir_lowering=False, debug=False,
                   enable_asserts=False, num_devices=N_CORES)

    xw_d = nc.dram_tensor("xw", [C, 2 * N], BF16, kind="ExternalInput")
    wqkT_d = nc.dram_tensor("wqkT", [C, 2 * C], BF16, kind="ExternalInput")
    wprojT_d = nc.dram_tensor("wprojT", [C, C], BF16, kind="ExternalInput")
    bias_d = nc.dram_tensor("bias", [P, CT], F32, kind="ExternalInput")
    ident_d = nc.dram_tensor("ident", [P, P], BF16, kind="ExternalInput")
    outT_d = nc.dram_tensor("outT", [C, N], BF16, kind="ExternalOutput")

    with tile.TileContext(nc) as tc:
        with (
            tc.tile_pool(name="res", bufs=1) as rp,
            tc.tile_pool(name="work", bufs=2) as wp,
            tc.tile_pool(name="ps", bufs=1, space="PSUM") as pp,
        ):
            # ---------------- PE warm-up ----------------
            # Cover the initial input-DMA wait with dummy matmuls so the
            # p-state/HAM ramp completes before real work arrives.
            warm_a = wp.tile([P, 512], BF16, name="warm_a", tag="warm_a",
                             bufs=1)
            nc.vector.memset(warm_a[:], 0.25)
            warm_ps = pp.tile([P, 512], F32, name="warm_ps", tag="acc",
                              bufs=1)
            for _ in range(5):
                nc.tensor.matmul(warm_ps[:], warm_a[:, 0:P], warm_a[:],
                                 start=True, stop=True)

            # ---------------- resident inputs ----------------
            # xw DMAs are split so the columns phase 1 needs (xT m0..3 +
            # wqv) land first; the xT m4..7 halves follow before phase 2.
            xT = []
            wqv = []
            xw_tiles = []
            for i in range(CT):
                t = rp.tile([P, 2 * N], BF16, name=f"xw{i}", tag=f"xw{i}")
                nc.sync.dma_start(t[:, 0:512],
                                  xw_d.ap()[i * P:(i + 1) * P, 0:512])
                nc.sync.dma_start(t[:, N:2 * N],
                                  xw_d.ap()[i * P:(i + 1) * P, N:2 * N])
                xw_tiles.append(t)
                xT.append(t[:, 0:N])
                wqv.append(t[:, N:2 * N])
            for i in range(CT):
                t = xw_tiles[i]
                nc.sync.dma_start(t[:, 512:N],
                                  xw_d.ap()[i * P:(i + 1) * P, 512:N])
            wqk = []
            for i in range(CT):
                t = rp.tile([P, 2 * C], BF16, name=f"wqk{i}", tag=f"wqk{i}")
                nc.sync.dma_start(t[:], wqkT_d.ap()[i * P:(i + 1) * P, :])
                wqk.append(t)
            wpj = []
            for i in range(CT):
                t = rp.tile([P, C], BF16, name=f"wpj{i}", tag=f"wpj{i}")
                nc.sync.dma_start(t[:], wprojT_d.ap()[i * P:(i + 1) * P, :])
                wpj.append(t)
            bias_t = rp.tile([P, CT], F32, name="bias_t", tag="bias")
            nc.sync.dma_start(bias_t[:], bias_d.ap())
            ident_t = rp.tile([P, P], BF16, name="ident_t", tag="ident")
            nc.sync.dma_start(ident_t[:], ident_d.ap())

            # ---------------- result tiles ----------------
            qT = [rp.tile([P, N], BF16, name=f"qT{i}", tag=f"qT{i}")
                  for i in range(PAIRS)]
            kT = [rp.tile([P, N], BF16, name=f"kT{i}", tag=f"kT{i}")
                  for i in range(PAIRS)]
            # vt[m]: [128 m-rows, 16 heads, 64 v-dims + ones col]
            vt = [rp.tile([P, H, D + 1], BF16, name=f"vt{m}", tag=f"vt{m}")
                  for m in range(MT)]
            ao = [rp.tile([P, N], BF16, name=f"ao{i}", tag=f"ao{i}")
                  for i in range(PAIRS)]

            for m in range(MT):
                nc.vector.memset(vt[m][:, :, D:D + 1], 1.0)

            # ---------------- emission helpers ----------------
            def vproj_m(m):
                """v-projection for m-tile m: [128 tokens, 1024 v-dims]."""
                ps = pp.tile([P, 1024], F32, name=f"vps{m}", tag="st", bufs=2)
                for j in range(2):
                    for c in range(CT):
                        nc.tensor.matmul(
                            ps[:, j * 512:(j + 1) * 512],
                            xT[c][:, m * P:(m + 1) * P],
                            wqv[c][:, j * 512:(j + 1) * 512],
                            start=(c == 0), stop=(c == CT - 1),
                        )
                nc.vector.tensor_copy(
                    vt[m][:, :, 0:D],
                    ps[:].rearrange("p (h d) -> p h d", d=D),
                )

            def qk_chain_mms(pr, which, n2, tag):
                """Returns (list of mm closures, finish closure)."""
                o0 = which * C + pr * P
                nsl = slice(n2 * 512, (n2 + 1) * 512)
                ps = pp.tile([P, 512], F32, name=f"qk{pr}_{which}_{n2}",
                             tag=tag, bufs=1)
                dst = (qT if which == 0 else kT)[pr]

                def mk(c):
                    def go():
                        nc.tensor.matmul(
                            ps[:],
                            wqk[c][:, o0:o0 + P],
                            xT[c][:, nsl],
                            start=(c == 0), stop=(c == CT - 1),
                        )
                    return go

                def fin():
                    nc.vector.tensor_copy(dst[:, nsl], ps[:])

                return [mk(c) for c in range(CT)], fin

            def s_pair(pr, n2, m):
                """Score matmuls for both heads of the pair + combined exp.
                Returns the pt tile."""
                nsl = slice(n2 * 512, (n2 + 1) * 512)
                msl = slice(m * P, (m + 1) * P)
                st_t = pp.tile([P, 1024], F32, name=f"st{pr}_{n2}_{m}",
                               tag="st", bufs=2)
                for h in range(2):
                    psl = slice(h * 64, (h + 1) * 64)
                    nc.tensor.matmul(
                        st_t[:, h * 512:(h + 1) * 512],
                        kT[pr][psl, msl],
                        qT[pr][psl, nsl],
                        start=True, stop=True,
                        tile_position=(h * 64, 0),
                    )
                pt_t = wp.tile([P, 1024], BF16, name=f"pt{pr}_{n2}_{m}",
                               tag="pt", bufs=18)
                nc.scalar.activation(pt_t[:], st_t[:], AF.Exp, scale=SCALE)
                return pt_t

            def av_chain(pr, n2, h, nu, pts, an_t):
                """Flipped AV for one head and one 128-col n-tile."""
                head = 2 * pr + h
                av_t = pp.tile([P, D + 1], F32, name=f"av{pr}_{n2}_{h}_{nu}",
                               tag="av", bufs=2)
                lo = h * 512 + nu * 128
                for mi in range(MT):
                    nc.tensor.matmul(
                        av_t[:],
                        pts[mi][:, lo:lo + 128],
                        vt[mi][:, head, :],
                        start=(mi == 0), stop=(mi == MT - 1),
                    )
                # normalize + evacuate: per-partition scale by 1/sums col
                rec = wp.tile([P, 1], F32, name=f"rc{pr}_{n2}_{h}_{nu}",
                              tag="rec", bufs=4)
                nc.vector.reciprocal(rec[:], av_t[:, D:D + 1])
                nc.vector.tensor_scalar_mul(
                    an_t[:, h * 64:(h + 1) * 64], av_t[:, 0:D], rec[:])

            partials = {}    # (n2, ot) -> bf16 partial (pr 0..3 sum + bias)

            def proj_lo_chain(n2, ot):
                """First-half output projection (pr 0..3) with bias folded
                in, parked to SBUF; runs in the late ACT-gated slots."""
                ps = pp.tile([P, 512], F32, name=f"ylo{n2}_{ot}", tag="acc",
                             bufs=1)
                nsl = slice(n2 * 512, (n2 + 1) * 512)
                for pr in range(4):
                    nc.tensor.matmul(
                        ps[:],
                        wpj[pr][:, ot * P:(ot + 1) * P],
                        ao[pr][:, nsl],
                        start=(pr == 0), stop=(pr == 3),
                    )
                pt_ = wp.tile([P, 512], BF16, name=f"ypart{n2}_{ot}",
                              tag="part", bufs=8)
                nc.vector.tensor_scalar_add(pt_[:], ps[:],
                                            bias_t[:, ot:ot + 1])
                partials[(n2, ot)] = pt_

            def proj_hi_chain(n2, ot, ps):
                """Second half (pr 4..7) + partial add, for parked tiles."""
                nsl = slice(n2 * 512, (n2 + 1) * 512)
                for pr in range(4, PAIRS):
                    nc.tensor.matmul(
                        ps[:],
                        wpj[pr][:, ot * P:(ot + 1) * P],
                        ao[pr][:, nsl],
                        start=(pr == 4), stop=(pr == PAIRS - 1),
                    )

                def fin():
                    yt = wp.tile([P, 512], BF16, name=f"yh{ot}_{n2}",
                                 tag="y", bufs=3)
                    nc.vector.tensor_add(yt[:], ps[:],
                                         partials[(n2, ot)][:])
                    nc.sync.dma_start(outT_d.ap()[ot * P:(ot + 1) * P, nsl],
                                      yt[:])

                return fin

            def proj_chain_mms(n2, ot, ps):
                """Output projection chain closures for tile (n2, ot)."""
                nsl = slice(n2 * 512, (n2 + 1) * 512)

                def mk(pr):
                    def go():
                        nc.tensor.matmul(
                            ps[:],
                            wpj[pr][:, ot * P:(ot + 1) * P],
                            ao[pr][:, nsl],
                            start=(pr == 0), stop=(pr == PAIRS - 1),
                        )
                    return go

                def fin():
                    yt = wp.tile([P, 512], BF16, name=f"y{ot}_{n2}", tag="y",
                                 bufs=3)
                    nc.vector.tensor_scalar_add(yt[:], ps[:],
                                                bias_t[:, ot:ot + 1])
                    nc.sync.dma_start(outT_d.ap()[ot * P:(ot + 1) * P, nsl],
                                      yt[:])

                return [mk(pr) for pr in range(PAIRS)], fin

            # ---------------- startup ----------------
            # Phase 1: vproj m0..m3 c-OUTER with 8 simultaneously-open psum
            # chains (all 8 banks) so each arriving xw c-tile feeds 8 matmuls
            # immediately -- the PE tracks the DMA stream instead of stalling
            # for the full 4MB xw tensor.
            ps01 = [pp.tile([P, 1024], F32, name=f"vps{m}", tag="st", bufs=2)
                    for m in range(2)]
            ps23 = {(2, 0): pp.tile([P, 512], F32, name="vp2a", tag="acc",
                                    bufs=1),
                    (2, 1): pp.tile([P, 512], F32, name="vp2b", tag="tr",
                                    bufs=1),
                    (3, 0): pp.tile([P, 512], F32, name="vp3a", tag="av",
                                    bufs=2),
                    (3, 1): pp.tile([P, 512], F32, name="vp3b", tag="av",
                                    bufs=2)}
            for c in range(CT):
                for m in range(4):
                    for j in range(2):
                        dst = (ps01[m][:, j * 512:(j + 1) * 512] if m < 2
                               else ps23[(m, j)][:])
                        nc.tensor.matmul(
                            dst,
                            xT[c][:, m * P:(m + 1) * P],
                            wqv[c][:, j * 512:(j + 1) * 512],
                            start=(c == 0), stop=(c == CT - 1),
                        )
            for m in range(2):
                nc.vector.tensor_copy(
                    vt[m][:, :, 0:D],
                    ps01[m][:].rearrange("p (h d) -> p h d", d=D))
            for m in (2, 3):
                for j in range(2):
                    nc.vector.tensor_copy(
                        vt[m][:, j * 8:(j + 1) * 8, 0:D],
                        ps23[(m, j)][:].rearrange("p (h d) -> p h d", d=D))

            # Phase 2: vproj m4..m7 as m-chains, then the pair-0 qk chains
            # (their wqk DMAs land only after all of xw).
            for m in range(4, MT):
                vproj_m(m)
            for which, n2, tag in ((0, 0, "acc"), (0, 1, "tr"),
                                   (1, 0, "acc"), (1, 1, "tr")):
                mms, fin = qk_chain_mms(0, which, n2, tag)
                for go in mms:
                    go()
                fin()

            # ---------------- main loop ----------------
            # Slot (pr, n2).  Filler work per slot:
            #  - previous slot's AV chains + divides + transposes + ao evac
            #  - next pair's q (n2=0 slot) / k (n2=1 slot) projection chains
            #  - vproj m6/m7 in slot (0,0); output projection at pr=7
            pts_prev = None      # (pr, n2, [pt tiles]) of previous slot

            for pr in range(PAIRS):
                for n2 in range(NT2):
                    slot = 2 * pr + n2
                    # --- gather filler: qk chains of next pair ---
                    qk_fill = []
                    if pr < PAIRS - 1:
                        which = n2          # q chains in n2=0, k in n2=1
                        qk_fill.append(
                            qk_chain_mms(pr + 1, which, 0, "acc"))
                        qk_fill.append(
                            qk_chain_mms(pr + 1, which, 1, "tr"))

                    # --- previous slot's AV work ---
                    if pts_prev is not None:
                        ppr, pn2, ppts = pts_prev
                        an_ts = [wp.tile([P, P], BF16,
                                         name=f"an{ppr}_{pn2}_{nu}",
                                         tag="an", bufs=6)
                                 for nu in range(4)]
                        tr_t = pp.tile([P, 512], BF16,
                                       name=f"tr{ppr}_{pn2}", tag="tr",
                                       bufs=1)

                        def mk_av(nu, h, _ppr=ppr, _pn2=pn2, _ppts=ppts,
                                  _an=an_ts):
                            def go():
                                av_chain(_ppr, _pn2, h, nu, _ppts, _an[nu])
                            return go

                        def mk_tr(nu, _an=an_ts, _tr=tr_t):
                            def go():
                                nc.tensor.transpose(
                                    _tr[:, nu * 128:(nu + 1) * 128],
                                    _an[nu][:], ident_t[:])
                            return go

                        def mk_evac(_ppr=ppr, _pn2=pn2, _tr=tr_t):
                            def go():
                                nc.vector.tensor_copy(
                                    ao[_ppr][:, _pn2 * 512:(_pn2 + 1) * 512],
                                    _tr[:])
                            return go

                        av_items = [mk_av(nu, h)
                                    for nu in range(4) for h in range(2)]
                        tr_items = [mk_tr(nu) for nu in range(4)]
                        evac_item = mk_evac()
                    else:
                        av_items, tr_items, evac_item = [], [], None

                    # --- weave the slot ---
                    # filler queue: list of closure-lists, consumed in order
                    # across the 8 m-steps.
                    fq = []
                    if pr == PAIRS - 1:
                        # ACT-gated last slots: fill with partial outproj
                        los = ([(0, 2), (0, 3), (0, 4), (0, 5)] if n2 == 0
                               else [(0, 6), (1, 0), (1, 1), (1, 2)])
                        for lo_n2, lo_ot in los:
                            fq.append([lambda a=lo_n2, b=lo_ot:
                                       proj_lo_chain(a, b)])
                    for mms, fin in qk_fill:
                        def qk_part(items):
                            def go():
                                for it in items:
                                    it()
                            return go
                        fq.append([qk_part(mms[0:4])])
                        fin_ = fin

                        def qk_rest(items=mms[4:8], f=fin_):
                            def go():
                                for it in items:
                                    it()
                                f()
                            return go
                        fq.append([qk_rest()])
                    # AV chains spread over mid/late m-steps, transposes after
                    av_sched = {3: av_items[0:2], 4: av_items[2:4],
                                5: av_items[4:6], 6: av_items[6:8]}
                    tr_sched = {5: tr_items[0:1], 6: tr_items[1:2],
                                7: tr_items[2:4]}

                    pts_now = []
                    for m in range(MT):
                        pts_now.append(s_pair(pr, n2, m))
                        if m < len(fq):
                            for it in fq[m]:
                                it()
                        for it in av_sched.get(m, []):
                            it()
                        for it in tr_sched.get(m, []):
                            it()
                        if m == MT - 1:
                            # leftover filler (slots with >8 filler groups)
                            for grp in fq[MT:]:
                                for it in grp:
                                    it()
                            if evac_item is not None:
                                evac_item()
                    pts_prev = (pr, n2, pts_now)

            # ---------------- tail ----------------
            # last slot's AV + transposes, then the rest of the projection
            ppr, pn2, ppts = pts_prev
            an_ts = [wp.tile([P, P], BF16, name=f"an{ppr}_{pn2}_{nu}",
                             tag="an", bufs=6) for nu in range(4)]
            tr_t = pp.tile([P, 512], BF16, name=f"tr{ppr}_{pn2}", tag="tr",
                           bufs=1)

            # fill the E(7,1,7) wait with two n2=0 proj chains on st halves
            st_tail = pp.tile([P, 1024], F32, name="st_tail", tag="st",
                              bufs=2)
            tail_fins = []
            for j, ot in enumerate((0, 1)):
                mms, fin = proj_chain_mms(0, ot,
                                          st_tail[:, j * 512:(j + 1) * 512])
                for go in mms[0:4]:
                    go()
                tail_fins.append((mms[4:], fin))

            for nu in range(4):
                for h in range(2):
                    av_chain(ppr, pn2, h, nu, ppts, an_ts[nu])
                if nu >= 1 and tail_fins:
                    mms, fin = tail_fins.pop(0)
                    for go in mms:
                        go()
                    fin()
            for mms, fin in tail_fins:
                for go in mms:
                    go()
                fin()
            for nu in range(4):
                nc.tensor.transpose(tr_t[:, nu * 128:(nu + 1) * 128],
                                    an_ts[nu][:], ident_t[:])
            nc.vector.tensor_copy(ao[ppr][:, pn2 * 512:(pn2 + 1) * 512],
                                  tr_t[:])

            # remaining projection: n2=0 tiles first (ao[7] n2=0 is ready a
            # slot earlier than n2=1); parked n2=1 hi-chains last -- the
            # final chain is short (4 matmuls) to minimize the drain tail.
            remaining = [(0, ot) for ot in range(2, CT)]
            remaining += [(1, ot) for ot in range(CT)]
            remaining.sort(key=lambda t: (t not in partials, t))
            tags = ["st2", "st2", "st3", "st3", "acc", "tr", "av", "av"]
            st2 = pp.tile([P, 1024], F32, name="st2", tag="st", bufs=2)
            st3 = pp.tile([P, 1024], F32, name="st3", tag="st", bufs=2)
            fins = []
            for i, (n2, ot) in enumerate(remaining):
                tg = tags[i % 8]
                if tg == "st2":
                    ps = st2[:, (i % 2) * 512:((i % 2) + 1) * 512]
                elif tg == "st3":
                    ps = st3[:, (i % 2) * 512:((i % 2) + 1) * 512]
                elif tg == "av":
                    ps = pp.tile([P, 512], F32, name=f"ytail{i}", tag="av",
                                 bufs=2)
                else:
                    ps = pp.tile([P, 512], F32, name=f"ytail{i}", tag=tg,
                                 bufs=1)
                last = (i == len(remaining) - 1)
                if last and (n2, ot) in partials:
                    # split the final parked tile into halves (separate psum
                    # banks) so the first half's add/DMA overlaps the second
                    # half's matmuls
                    ps2 = pp.tile([P, 256], F32, name="yfin_ps2", tag="av",
                                  bufs=2)
                    for half in range(2):
                        pst = ps if half == 0 else ps2
                        csl = slice(half * 256, (half + 1) * 256)
                        asl = slice(n2 * 512 + half * 256,
                                    n2 * 512 + (half + 1) * 256)
                        for pr in range(4, PAIRS):
                            nc.tensor.matmul(
                                pst[:, 0:256],
                                wpj[pr][:, ot * P:(ot + 1) * P],
                                ao[pr][:, asl],
                                start=(pr == 4), stop=(pr == PAIRS - 1),
                            )
                        yt = wp.tile([P, 256], BF16, name=f"yfin{half}",
                                     tag="y", bufs=3)
                        nc.vector.tensor_add(yt[:], pst[:, 0:256],
                                             partials[(n2, ot)][:, csl])
                        nc.sync.dma_start(
                            outT_d.ap()[ot * P:(ot + 1) * P, asl], yt[:])
                        if half == 0:
                            for fin in fins:
                                fin()
                            fins = []
                elif (n2, ot) in partials:
                    fins.append(proj_hi_chain(n2, ot, ps))
                    if len(fins) >= 2:
                        fins.pop(0)()
                elif not last:
                    mms, fin = proj_chain_mms(n2, ot, ps)
                    for go in mms:
                        go()
                    fins.append(fin)
                    # drain finishes with one-chain delay so bufs recycle
                    if len(fins) >= 2:
                        fins.pop(0)()
                else:
                    # split the last output tile into halves (separate psum
                    # banks -- a shared bank would serialize half1's start
                    # behind half0's bias-add read via the zero region) so
                    # the first bias-add/DMA overlaps the second half's mms
                    ps2 = pp.tile([P, 256], F32, name="ylast_ps2", tag="av",
                                  bufs=2)
                    for half in range(2):
                        pst = ps if half == 0 else ps2
                        for pr in range(PAIRS):
                            nc.tensor.matmul(
                                pst[:, 0:256],
                                wpj[pr][:, ot * P:(ot + 1) * P],
                                ao[pr][:, n2 * 512 + half * 256:
                                       n2 * 512 + (half + 1) * 256],
                                start=(pr == 0), stop=(pr == PAIRS - 1),
                            )
                        yt = wp.tile([P, 256], BF16, name=f"ylast{half}",
                                     tag="y", bufs=3)
                        nc.vector.tensor_scalar_add(yt[:], pst[:, 0:256],
                                                    bias_t[:, ot:ot + 1])
                        nc.sync.dma_start(
                            outT_d.ap()[ot * P:(ot + 1) * P,
                                        n2 * 512 + half * 256:
                                        n2 * 512 + (half + 1) * 256],
                            yt[:])
                        if half == 0:
                            for fin in fins:
                                fin()
                            fins = []
            for fin in fins:
                fin()

    nc.compile()
    return nc


def get_nc():
    if "nc" not in _cache:
        _cache["nc"] = _build()
    return _cache["nc"]


def kernel(x, w_qkv, w_proj, b_proj):
    x = np.asarray(x, dtype=np.float32)
    w_qkv = np.asarray(w_qkv, dtype=np.float32)
    w_proj = np.asarray(w_proj, dtype=np.float32)
    b_proj = np.asarray(b_proj, dtype=np.float32)

    bf = ml_dtypes.bfloat16
    wqkvT = np.ascontiguousarray(w_qkv.T).astype(bf)     # [C, 3C]
    wqkT = np.ascontiguousarray(wqkvT[:, 0:2 * C])       # [C, 2C] q,k cols
    wprojT = np.ascontiguousarray(w_proj.T).astype(bf)   # [C, C]
    bias = np.ascontiguousarray(b_proj.reshape(CT, P).T).astype(np.float32)
    ident = np.eye(P, dtype=bf)

    in_maps = []
    wqv_host = wqkvT[:, 2 * C:]                          # [C, C] v columns
    for b in range(N_CORES):
        xT = np.ascontiguousarray(x[b].T).astype(bf)     # [C, N]
        xw = np.ascontiguousarray(np.concatenate([xT, wqv_host], axis=1))
        in_maps.append({"xw": xw, "wqkT": wqkT, "wprojT": wprojT,
                        "bias": bias, "ident": ident})

    nc = get_nc()
    _cache["in_maps"] = in_maps
    res = bass_utils.run_bass_kernel_spmd(nc, in_maps,
                                          core_ids=list(range(N_CORES)))
    out = np.empty((B, N, C), dtype=np.float32)
    for b in range(N_CORES):
        out[b] = res.results[b]["outT"].T.astype(np.float32)
    return out


# revision 56
# speedup vs baseline: 1.0090x; 1.0045x over previous
"""Trainium2 Bass kernel for batched multi-head self-attention.

Reference computation (per batch element b):
    qkv = x @ w_qkv.T                  # [N, 3C]
    q, k, v = split/reshape to heads   # H=16 heads, d=64
    attn = softmax(q @ k.T / sqrt(d))
    out = (attn @ v) reshaped back     # [N, C]
    y = out @ w_proj.T + b_proj

Sharding: pure data-parallel over batch B=8 across the 8 NeuronCores
(one batch element per core, weights replicated, no collectives).

On-device layout (everything transposed so matmuls contract over the
partition axis with no input transposes):
  - xT      [C, N]   (host pre-transposed, bf16)
  - wqkT    [C, 2C]  (host pre-transposed q,k columns, bf16)
  - wprojT  [C, C]   (host pre-transposed, bf16)

Cost model: a PE matmul costs out_free_size * 0.4167ns regardless of
contraction size or output partitions.  The key restructure vs the
229us baseline is the attn@V matmul orientation:
  - scores S^T [m,n] tiles (K=64, head pairs row-packed): free=512.
  - AV is computed FLIPPED: out[n, d+1] = pt[m, n-tile].T @ [V | 1]
    with free=65 instead of the old [d+1, 512] free=512 layout.
    This halves AV PE time (54.6us -> 27.7us).  The ones column gives
    softmax row sums per output partition, so normalization is a
    per-partition reciprocal + tensor_scalar_mul on DVE (no partition
    broadcasts).
  - The [n, c] result is transposed back to [c, n] for the output
    projection with PE transpose instructions (128x128 via identity,
    53ns each, 64 total = 3.4us).
  - exp() is done on [128, 1024] head-pair tiles (one ACT instruction
    per (n2, m)) to amortize the ~370ns ACT access-latency overhead;
    ACT total 133us stays under the PE's ~197us.
Schedule (TimelineSim engines execute strictly in emission order):
  - Startup: vproj m0..3 run c-OUTER with 8 open psum chains so each
    arriving xw c-tile feeds 8 matmuls at once (PE tracks the DMA
    stream; xw DMAs are split so those columns land first), then
    vproj m4..7 + pair-0 qk chains while the wqk DMAs land.
  - Each attention slot (pr, n2) weaves its 8 score-pairs/exps with
    the previous slot's AV chains + transposes and the next pair's
    q/k projection chains, so the PE never waits on ACT.
  - The ACT-gated last two slots are filled with parked half output
    projection chains (pr 0..3, bias folded in); the tail finishes
    them (pr 4..7 + one DVE add each) and the remaining full chains,
    splitting the last tile across two psum banks to overlap its
    bias/DMA with the final matmuls.
PSUM budget (8 banks): st 2x2 + acc 1 + av 2 + tr 1.
The y staging tiles use 6 bufs: with fewer, a bias-add stalls on a
buffer whose previous DMA read completes only at transfer end + 900ns
(SEM_PROP_DMA_OVERHEAD) -- a WAR hazard worth ~0.8us at the tail.
TimelineSim: 205.7us (baseline 229.2us); PE busy 197.2us (95.9%).
"""

import os
import sys

for _p in ("/opt/trn_rl_repo", "/root/.axon_site/_ro/trn_rl_repo"):
    if os.path.isdir(_p) and _p not in sys.path:
        sys.path.insert(0, _p)
        break

import numpy as np
import ml_dtypes

import concourse.bass as bass
import concourse.bacc as bacc
import concourse.tile as tile
import concourse.mybir as mybir
from concourse import bass_utils

BF16 = mybir.dt.bfloat16
F32 = mybir.dt.float32
AF = mybir.ActivationFunctionType
ALU = mybir.AluOpType

B, N, C, H = 8, 1024, 1024, 16
D = C // H            # 64 head dim
P = 128               # partitions
CT = C // P           # 8 contraction tiles
NT2 = N // 512        # 2 n-tiles of 512
MT = N // P           # 8 m-tiles of 128
PAIRS = H // 2        # 8 head pairs
SCALE = float(D) ** -0.5
N_CORES = 8

_cache = {}


def _build():
    nc = bacc.Bacc("TRN2", target_bir_lowering=False, debug=False,
                   enable_asserts=False, num_devices=N_CORES)

    xw_d = nc.dram_tensor("xw", [C, 2 * N], BF16, kind="ExternalInput")
    wqkT_d = nc.dram_tensor("wqkT", [C, 2 * C], BF16, kind="ExternalInput")
    wprojT_d = nc.dram_tensor("wprojT", [C, C], BF16, kind="ExternalInput")
    bias_d = nc.dram_tensor("bias", [P, CT], F32, kind="ExternalInput")
    ident_d = nc.dram_tensor("ident", [P, P], BF16, kind="ExternalInput")
    outT_d = nc.dram_tensor("outT", [C, N], BF16, kind="ExternalOutput")

    with tile.TileContext(nc) as tc:
        with (
            tc.tile_pool(name="res", bufs=1) as rp,
            tc.tile_pool(name="work", bufs=2) as wp,
            tc.tile_pool(name="ps", bufs=1, space="PSUM") as pp,
        ):
            # ---------------- PE warm-up ----------------
            # Cover the initial input-DMA wait with dummy matmuls so the
            # p-state/HAM ramp completes before real work arrives.
            warm_a = wp.tile([P, 512], BF16, name="warm_a", tag="warm_a",
                             bufs=1)
            nc.vector.memset(warm_a[:], 0.25)
            warm_ps = pp.tile([P, 512], F32, name="warm_ps", tag="acc",
                              bufs=1)
            for _ in range(5):
                nc.tensor.matmul(warm_ps[:], warm_a[:, 0:P], warm_a[:],
                                 start=True, stop=True)

            # ---------------- resident inputs ----------------
            # xw DMAs are split so the columns phase 1 needs (xT m0..3 +
            # wqv) land first; the xT m4..7 halves follow before phase 2.
            xT = []
            wqv = []
            xw_tiles = []
            for i in range(CT):
                t = rp.tile([P, 2 * N], BF16, name=f"xw{i}", tag=f"xw{i}")
                nc.sync.dma_start(t[:, 0:512],
                                  xw_d.ap()[i * P:(i + 1) * P, 0:512])
                nc.sync.dma_start(t[:, N:2 * N],
                                  xw_d.ap()[i * P:(i + 1) * P, N:2 * N])
                xw_tiles.append(t)
                xT.append(t[:, 0:N])
                wqv.append(t[:, N:2 * N])
            for i in range(CT):
                t = xw_tiles[i]
                nc.sync.dma_start(t[:, 512:N],
                                  xw_d.ap()[i * P:(i + 1) * P, 512:N])
            wqk = []
            for i in range(CT):
                t = rp.tile([P, 2 * C], BF16, name=f"wqk{i}", tag=f"wqk{i}")
                nc.sync.dma_start(t[:], wqkT_d.ap()[i * P:(i + 1) * P, :])
                wqk.append(t)
            wpj = []
            for i in range(CT):
                t = rp.tile([P, C], BF16, name=f"wpj{i}", tag=f"wpj{i}")
                nc.sync.dma_start(t[:], wprojT_d.ap()[i * P:(i + 1) * P, :])
                wpj.append(t)
            bias_t = rp.tile([P, CT], F32, name="bias_t", tag="bias")
            nc.sync.dma_start(bias_t[:], bias_d.ap())
            ident_t = rp.tile([P, P], BF16, name="ident_t", tag="ident")
            nc.sync.dma_start(ident_t[:], ident_d.ap())

            # ---------------- result tiles ----------------
            qT = [rp.tile([P, N], BF16, name=f"qT{i}", tag=f"qT{i}")
                  for i in range(PAIRS)]
            kT = [rp.tile([P, N], BF16, name=f"kT{i}", tag=f"kT{i}")
                  for i in range(PAIRS)]
            # vt[m]: [128 m-rows, 16 heads, 64 v-dims + ones col]
            vt = [rp.tile([P, H, D + 1], BF16, name=f"vt{m}", tag=f"vt{m}")
                  for m in range(MT)]
            ao = [rp.tile([P, N], BF16, name=f"ao{i}", tag=f"ao{i}")
                  for i in range(PAIRS)]

            for m in range(MT):
                nc.vector.memset(vt[m][:, :, D:D + 1], 1.0)

            # ---------------- emission helpers ----------------
            def vproj_m(m):
                """v-projection for m-tile m: [128 tokens, 1024 v-dims]."""
                ps = pp.tile([P, 1024], F32, name=f"vps{m}", tag="st", bufs=2)
                for j in range(2):
                    for c in range(CT):
                        nc.tensor.matmul(
                            ps[:, j * 512:(j + 1) * 512],
                            xT[c][:, m * P:(m + 1) * P],
                            wqv[c][:, j * 512:(j + 1) * 512],
                            start=(c == 0), stop=(c == CT - 1),
                        )
                nc.vector.tensor_copy(
                    vt[m][:, :, 0:D],
                    ps[:].rearrange("p (h d) -> p h d", d=D),
                )

            def qk_chain_mms(pr, which, n2, tag):
                """Returns (list of mm closures, finish closure)."""
                o0 = which * C + pr * P
                nsl = slice(n2 * 512, (n2 + 1) * 512)
                ps = pp.tile([P, 512], F32, name=f"qk{pr}_{which}_{n2}",
                             tag=tag, bufs=1)
                dst = (qT if which == 0 else kT)[pr]

                def mk(c):
                    def go():
                        nc.tensor.matmul(
                            ps[:],
                            wqk[c][:, o0:o0 + P],
                            xT[c][:, nsl],
                            start=(c == 0), stop=(c == CT - 1),
                        )
                    return go

                def fin():
                    nc.vector.tensor_copy(dst[:, nsl], ps[:])

                return [mk(c) for c in range(CT)], fin

            def s_pair(pr, n2, m):
                """Score matmuls for both heads of the pair + combined exp.
                Returns the pt tile."""
                nsl = slice(n2 * 512, (n2 + 1) * 512)
                msl = slice(m * P, (m + 1) * P)
                st_t = pp.tile([P, 1024], F32, name=f"st{pr}_{n2}_{m}",
                               tag="st", bufs=2)
                for h in range(2):
                    psl = slice(h * 64, (h + 1) * 64)
                    nc.tensor.matmul(
                        st_t[:, h * 512:(h + 1) * 512],
                        kT[pr][psl, msl],
                        qT[pr][psl, nsl],
                        start=True, stop=True,
                        tile_position=(h * 64, 0),
                    )
                pt_t = wp.tile([P, 1024], BF16, name=f"pt{pr}_{n2}_{m}",
                               tag="pt", bufs=18)
                nc.scalar.activation(pt_t[:], st_t[:], AF.Exp, scale=SCALE)
                return pt_t

            def av_chain(pr, n2, h, nu, pts, an_t):
                """Flipped AV for one head and one 128-col n-tile."""
                head = 2 * pr + h
                av_t = pp.tile([P, D + 1], F32, name=f"av{pr}_{n2}_{h}_{nu}",
                               tag="av", bufs=2)
                lo = h * 512 + nu * 128
                for mi in range(MT):
                    nc.tensor.matmul(
                        av_t[:],
                        pts[mi][:, lo:lo + 128],
                        vt[mi][:, head, :],
                        start=(mi == 0), stop=(mi == MT - 1),
                    )
                # normalize + evacuate: per-partition scale by 1/sums col
                rec = wp.tile([P, 1], F32, name=f"rc{pr}_{n2}_{h}_{nu}",
                              tag="rec", bufs=4)
                nc.vector.reciprocal(rec[:], av_t[:, D:D + 1])
                nc.vector.tensor_scalar_mul(
                    an_t[:, h * 64:(h + 1) * 64], av_t[:, 0:D], rec[:])

            partials = {}    # (n2, ot) -> bf16 partial (pr 0..3 sum + bias)

            def proj_lo_chain(n2, ot):
                """First-half output projection (pr 0..3) with bias folded
                in, parked to SBUF; runs in the late ACT-gated slots."""
                ps = pp.tile([P, 512], F32, name=f"ylo{n2}_{ot}", tag="acc",
                             bufs=1)
                nsl = slice(n2 * 512, (n2 + 1) * 512)
                for pr in range(4):
                    nc.tensor.matmul(
                        ps[:],
                        wpj[pr][:, ot * P:(ot + 1) * P],
                        ao[pr][:, nsl],
                        start=(pr == 0), stop=(pr == 3),
                    )
                pt_ = wp.tile([P, 512], BF16, name=f"ypart{n2}_{ot}",
                              tag="part", bufs=8)
                nc.vector.tensor_scalar_add(pt_[:], ps[:],
                                            bias_t[:, ot:ot + 1])
                partials[(n2, ot)] = pt_

            def proj_hi_chain(n2, ot, ps):
                """Second half (pr 4..7) + partial add, for parked tiles."""
                nsl = slice(n2 * 512, (n2 + 1) * 512)
                for pr in range(4, PAIRS):
                    nc.tensor.matmul(
                        ps[:],
                        wpj[pr][:, ot * P:(ot + 1) * P],
                        ao[pr][:, nsl],
                        start=(pr == 4), stop=(pr == PAIRS - 1),
                    )

                def fin():
                    yt = wp.tile([P, 512], BF16, name=f"yh{ot}_{n2}",
                                 tag="y", bufs=6)
                    nc.vector.tensor_add(yt[:], ps[:],
                                         partials[(n2, ot)][:])
                    nc.sync.dma_start(outT_d.ap()[ot * P:(ot + 1) * P, nsl],
                                      yt[:])

                return fin

            def proj_chain_mms(n2, ot, ps):
                """Output projection chain closures for tile (n2, ot)."""
                nsl = slice(n2 * 512, (n2 + 1) * 512)

                def mk(pr):
                    def go():
                        nc.tensor.matmul(
                            ps[:],
                            wpj[pr][:, ot * P:(ot + 1) * P],
                            ao[pr][:, nsl],
                            start=(pr == 0), stop=(pr == PAIRS - 1),
                        )
                    return go

                def fin():
                    yt = wp.tile([P, 512], BF16, name=f"y{ot}_{n2}", tag="y",
                                 bufs=6)
                    nc.vector.tensor_scalar_add(yt[:], ps[:],
                                                bias_t[:, ot:ot + 1])
                    nc.sync.dma_start(outT_d.ap()[ot * P:(ot + 1) * P, nsl],
                                      yt[:])

                return [mk(pr) for pr in range(PAIRS)], fin

            # ---------------- startup ----------------
            # Phase 1: vproj m0..m3 c-OUTER with 8 simultaneously-open psum
            # chains (all 8 banks) so each arriving xw c-tile feeds 8 matmuls
            # immediately -- the PE tracks the DMA stream instead of stalling
            # for the full 4MB xw tensor.
            ps01 = [pp.tile([P, 1024], F32, name=f"vps{m}", tag="st", bufs=2)
                    for m in range(2)]
            ps23 = {(2, 0): pp.tile([P, 512], F32, name="vp2a", tag="acc",
                                    bufs=1),
                    (2, 1): pp.tile([P, 512], F32, name="vp2b", tag="tr",
                                    bufs=1),
                    (3, 0): pp.tile([P, 512], F32, name="vp3a", tag="av",
                                    bufs=2),
                    (3, 1): pp.tile([P, 512], F32, name="vp3b", tag="av",
                                    bufs=2)}
            for c in range(CT):
                for m in range(4):
                    for j in range(2):
                        dst = (ps01[m][:, j * 512:(j + 1) * 512] if m < 2
                               else ps23[(m, j)][:])
                        nc.tensor.matmul(
                            dst,
                            xT[c][:, m * P:(m + 1) * P],
                            wqv[c][:, j * 512:(j + 1) * 512],
                            start=(c == 0), stop=(c == CT - 1),
                        )
            for m in range(2):
                nc.vector.tensor_copy(
                    vt[m][:, :, 0:D],
                    ps01[m][:].rearrange("p (h d) -> p h d", d=D))
            for m in (2, 3):
                for j in range(2):
                    nc.vector.tensor_copy(
                        vt[m][:, j * 8:(j + 1) * 8, 0:D],
                        ps23[(m, j)][:].rearrange("p (h d) -> p h d", d=D))

            # Phase 2: vproj m4..m7 as m-chains, then the pair-0 qk chains
            # (their wqk DMAs land only after all of xw).
            for m in range(4, MT):
                vproj_m(m)
            for which, n2, tag in ((0, 0, "acc"), (0, 1, "tr"),
                                   (1, 0, "acc"), (1, 1, "tr")):
                mms, fin = qk_chain_mms(0, which, n2, tag)
                for go in mms:
                    go()
                fin()

            # ---------------- main loop ----------------
            # Slot (pr, n2).  Filler work per slot:
            #  - previous slot's AV chains + divides + transposes + ao evac
            #  - next pair's q (n2=0 slot) / k (n2=1 slot) projection chains
            #  - vproj m6/m7 in slot (0,0); output projection at pr=7
            pts_prev = None      # (pr, n2, [pt tiles]) of previous slot

            for pr in range(PAIRS):
                for n2 in range(NT2):
                    slot = 2 * pr + n2
                    # --- gather filler: qk chains of next pair ---
                    qk_fill = []
                    if pr < PAIRS - 1:
                        which = n2          # q chains in n2=0, k in n2=1
                        qk_fill.append(
                            qk_chain_mms(pr + 1, which, 0, "acc"))
                        qk_fill.append(
                            qk_chain_mms(pr + 1, which, 1, "tr"))

                    # --- previous slot's AV work ---
                    if pts_prev is not None:
                        ppr, pn2, ppts = pts_prev
                        an_ts = [wp.tile([P, P], BF16,
                                         name=f"an{ppr}_{pn2}_{nu}",
                                         tag="an", bufs=6)
                                 for nu in range(4)]
                        tr_t = pp.tile([P, 512], BF16,
                                       name=f"tr{ppr}_{pn2}", tag="tr",
                                       bufs=1)

                        def mk_av(nu, h, _ppr=ppr, _pn2=pn2, _ppts=ppts,
                                  _an=an_ts):
                            def go():
                                av_chain(_ppr, _pn2, h, nu, _ppts, _an[nu])
                            return go

                        def mk_tr(nu, _an=an_ts, _tr=tr_t):
                            def go():
                                nc.tensor.transpose(
                                    _tr[:, nu * 128:(nu + 1) * 128],
                                    _an[nu][:], ident_t[:])
                            return go

                        def mk_evac(_ppr=ppr, _pn2=pn2, _tr=tr_t):
                            def go():
                                nc.vector.tensor_copy(
                                    ao[_ppr][:, _pn2 * 512:(_pn2 + 1) * 512],
                                    _tr[:])
                            return go

                        av_items = [mk_av(nu, h)
                                    for nu in range(4) for h in range(2)]
                        tr_items = [mk_tr(nu) for nu in range(4)]
                        evac_item = mk_evac()
                    else:
                        av_items, tr_items, evac_item = [], [], None

                    # --- weave the slot ---
                    # filler queue: list of closure-lists, consumed in order
                    # across the 8 m-steps.
                    fq = []
                    if pr == PAIRS - 1:
                        # ACT-gated last slots: fill with partial outproj
                        # (skip m=0: their acc buf frees only after the
                        # previous slot's last VE copy lands)
                        fq.append([])
                        los = ([(0, 2), (0, 3), (0, 4), (0, 5)] if n2 == 0
                               else [(0, 6), (1, 0), (1, 1), (1, 2)])
                        for lo_n2, lo_ot in los:
                            fq.append([lambda a=lo_n2, b=lo_ot:
                                       proj_lo_chain(a, b)])
                    for mms, fin in qk_fill:
                        def qk_part(items):
                            def go():
                                for it in items:
                                    it()
                            return go
                        fq.append([qk_part(mms[0:4])])
                        fin_ = fin

                        def qk_rest(items=mms[4:8], f=fin_):
                            def go():
                                for it in items:
                                    it()
                                f()
                            return go
                        fq.append([qk_rest()])
                    # AV chains spread over mid/late m-steps, transposes after
                    av_sched = {3: av_items[0:2], 4: av_items[2:4],
                                5: av_items[4:6], 6: av_items[6:8]}
                    tr_sched = {5: tr_items[0:1], 6: tr_items[1:2],
                                7: tr_items[2:4]}

                    pts_now = []
                    for m in range(MT):
                        pts_now.append(s_pair(pr, n2, m))
                        if m < len(fq):
                            for it in fq[m]:
                                it()
                        for it in av_sched.get(m, []):
                            it()
                        for it in tr_sched.get(m, []):
                            it()
                        if m == MT - 1:
                            # leftover filler (slots with >8 filler groups)
                            for grp in fq[MT:]:
                                for it in grp:
                                    it()
                            if evac_item is not None:
                                evac_item()
                    pts_prev = (pr, n2, pts_now)

            # ---------------- tail ----------------
            # last slot's AV + transposes, then the rest of the projection
            ppr, pn2, ppts = pts_prev
            an_ts = [wp.tile([P, P], BF16, name=f"an{ppr}_{pn2}_{nu}",
                             tag="an", bufs=6) for nu in range(4)]
            tr_t = pp.tile([P, 512], BF16, name=f"tr{ppr}_{pn2}", tag="tr",
                           bufs=1)

            # fill the E(7,1,7) wait with two n2=0 proj chains on st halves
            st_tail = pp.tile([P, 1024], F32, name="st_tail", tag="st",
                              bufs=2)
            tail_fins = []
            for j, ot in enumerate((0, 1)):
                mms, fin = proj_chain_mms(0, ot,
                                          st_tail[:, j * 512:(j + 1) * 512])
                for go in mms[0:4]:
                    go()
                tail_fins.append((mms[4:], fin))

            for nu in range(4):
                for h in range(2):
                    av_chain(ppr, pn2, h, nu, ppts, an_ts[nu])
                if nu >= 1 and tail_fins:
                    mms, fin = tail_fins.pop(0)
                    for go in mms:
                        go()
                    fin()
            for mms, fin in tail_fins:
                for go in mms:
                    go()
                fin()
            for nu in range(4):
                nc.tensor.transpose(tr_t[:, nu * 128:(nu + 1) * 128],
                                    an_ts[nu][:], ident_t[:])
            nc.vector.tensor_copy(ao[ppr][:, pn2 * 512:(pn2 + 1) * 512],
                                  tr_t[:])

            # remaining projection: n2=0 tiles first (ao[7] n2=0 is ready a
            # slot earlier than n2=1); parked n2=1 hi-chains last -- the
            # final chain is short (4 matmuls) to minimize the drain tail.
            remaining = [(0, ot) for ot in range(2, CT)]
            remaining += [(1, ot) for ot in range(CT)]
            remaining.sort(key=lambda t: (t not in partials, t))
            tags = ["st2", "st2", "st3", "st3", "acc", "tr", "av", "av"]
            st2 = pp.tile([P, 1024], F32, name="st2", tag="st", bufs=2)
            st3 = pp.tile([P, 1024], F32, name="st3", tag="st", bufs=2)
            fins = []
            for i, (n2, ot) in enumerate(remaining):
                tg = tags[i % 8]
                if tg == "st2":
                    ps = st2[:, (i % 2) * 512:((i % 2) + 1) * 512]
                elif tg == "st3":
                    ps = st3[:, (i % 2) * 512:((i % 2) + 1) * 512]
                elif tg == "av":
                    ps = pp.tile([P, 512], F32, name=f"ytail{i}", tag="av",
                                 bufs=2)
                else:
                    ps = pp.tile([P, 512], F32, name=f"ytail{i}", tag=tg,
                                 bufs=1)
                last = (i == len(remaining) - 1)
                if last and (n2, ot) in partials:
                    # split the final parked tile into halves (separate psum
                    # banks) so the first half's add/DMA overlaps the second
                    # half's matmuls
                    ps2 = pp.tile([P, 256], F32, name="yfin_ps2", tag="av",
                                  bufs=2)
                    for half in range(2):
                        pst = ps if half == 0 else ps2
                        csl = slice(half * 256, (half + 1) * 256)
                        asl = slice(n2 * 512 + half * 256,
                                    n2 * 512 + (half + 1) * 256)
                        for pr in range(4, PAIRS):
                            nc.tensor.matmul(
                                pst[:, 0:256],
                                wpj[pr][:, ot * P:(ot + 1) * P],
                                ao[pr][:, asl],
                                start=(pr == 4), stop=(pr == PAIRS - 1),
                            )
                        yt = wp.tile([P, 256], BF16, name=f"yfin{half}",
                                     tag="y", bufs=6)
                        nc.vector.tensor_add(yt[:], pst[:, 0:256],
                                             partials[(n2, ot)][:, csl])
                        nc.sync.dma_start(
                            outT_d.ap()[ot * P:(ot + 1) * P, asl], yt[:])
                        if half == 0:
                            for fin in fins:
                                fin()
                            fins = []
                elif (n2, ot) in partials:
                    fins.append(proj_hi_chain(n2, ot, ps))
                    if len(fins) >= 2:
                        fins.pop(0)()
                elif not last:
                    mms, fin = proj_chain_mms(n2, ot, ps)
                    for go in mms:
                        go()
                    fins.append(fin)
                    # drain finishes with one-chain delay so bufs recycle
                    if len(fins) >= 2:
                        fins.pop(0)()
                else:
                    # split the last output tile into halves (separate psum
                    # banks -- a shared bank would serialize half1's start
                    # behind half0's bias-add read via the zero region) so
                    # the first bias-add/DMA overlaps the second half's mms
                    ps2 = pp.tile([P, 256], F32, name="ylast_ps2", tag="av",
                                  bufs=2)
                    for half in range(2):
                        pst = ps if half == 0 else ps2
                        for pr in range(PAIRS):
                            nc.tensor.matmul(
                                pst[:, 0:256],
                                wpj[pr][:, ot * P:(ot + 1) * P],
                                ao[pr][:, n2 * 512 + half * 256:
                                       n2 * 512 + (half + 1) * 256],
                                start=(pr == 0), stop=(pr == PAIRS - 1),
                            )
                        if half == 0:
                            # drain pending fins BEFORE the final bias-adds
                            # so they don't delay the last store on the
                            # in-order VE queue
                            for fin in fins:
                                fin()
                            fins = []
                        yt = wp.tile([P, 256], BF16, name=f"ylast{half}",
                                     tag="y", bufs=6)
                        nc.vector.tensor_scalar_add(yt[:], pst[:, 0:256],
                                                    bias_t[:, ot:ot + 1])
                        nc.sync.dma_start(
                            outT_d.ap()[ot * P:(ot + 1) * P,
                                        n2 * 512 + half * 256:
                                        n2 * 512 + (half + 1) * 256],
                            yt[:])
            for fin in fins:
                fin()

    nc.compile()
    return nc


def get_nc():
    if "nc" not in _cache:
        _cache["nc"] = _build()
    return _cache["nc"]


def kernel(x, w_qkv, w_proj, b_proj):
    x = np.asarray(x, dtype=np.float32)
    w_qkv = np.asarray(w_qkv, dtype=np.float32)
    w_proj = np.asarray(w_proj, dtype=np.float32)
    b_proj = np.asarray(b_proj, dtype=np.float32)

    bf = ml_dtypes.bfloat16
    wqkvT = np.ascontiguousarray(w_qkv.T).astype(bf)     # [C, 3C]
    wqkT = np.ascontiguousarray(wqkvT[:, 0:2 * C])       # [C, 2C] q,k cols
    wprojT = np.ascontiguousarray(w_proj.T).astype(bf)   # [C, C]
    bias = np.ascontiguousarray(b_proj.reshape(CT, P).T).astype(np.float32)
    ident = np.eye(P, dtype=bf)

    in_maps = []
    wqv_host = wqkvT[:, 2 * C:]                          # [C, C] v columns
    for b in range(N_CORES):
        xT = np.ascontiguousarray(x[b].T).astype(bf)     # [C, N]
        xw = np.ascontiguousarray(np.concatenate([xT, wqv_host], axis=1))
        in_maps.append({"xw": xw, "wqkT": wqkT, "wprojT": wprojT,
                        "bias": bias, "ident": ident})

    nc = get_nc()
    _cache["in_maps"] = in_maps
    res = bass_utils.run_bass_kernel_spmd(nc, in_maps,
                                          core_ids=list(range(N_CORES)))
    out = np.empty((B, N, C), dtype=np.float32)
    for b in range(N_CORES):
        out[b] = res.results[b]["outT"].T.astype(np.float32)
    return out
